# revision 29
# baseline (speedup 1.0000x reference)
"""Block-causal attention block (RMSnorm + QKV + frame-causal attention + proj)
on 8 TRN2 NeuronCores — fp8 DoubleRow + weight-folding edition.

Sharding: as the baseline — core j (p=j//2, h=j%2) owns query half-blocks
(frame p, col-half h) and (frame 7-p, col-half h); each core runs a uniform
stream of 18 (kv half-block, q-half) pair-steps (perfectly balanced since
2(p+1) + 2(8-p) = 18).

Algebraic folds (vs the baseline's per-step K/V projections):
  - K-fold: scores s[kv,q] = x_raw[:,kv]^T (Wk_fold q) — the K projection
    moves to the query side (one GEMM per q-half); the k-bias term is
    constant per query row and drops by softmax shift invariance.
  - V-fold: O = Wv_fold (sum_kv xn[:,kv] p[kv,q]) — the V projection moves
    to after the attention sum (one GEMM per q-half); bv folds through wp
    into the output bias since softmax rows sum to 1.
  - Norm-fold: RMS norm scalar rho[t] = sqrt(C)/||x_t|| is computed on the
    DVE (free-axis square-reduce of the transposed slab + Quake rsqrt) and
    applied as (a) the per-partition scale AP of the Exp activation on the
    kv side and (b) a broadcast row multiply on the q side. gamma folds
    into the weights host-side.

All matmuls run in fp8 e4m3 with perf_mode=DoubleRow (K=256 per
instruction, 2 fp8 MACs/cell/cycle). Weights are pre-scaled by 64 (16 for
Wv) host-side to sit in e4m3's normal range; the compensations fold into
the exp scale and two output-side constants. Residual x + bias stays f32.

Per-pair-step PE work: 8 DR matmuls (scores) + 8 (U accum) + 2 (den)
vs the baseline's 64+8 full-rate f32r matmuls per step.
"""

import sys

import numpy as np
import ml_dtypes

sys.path.insert(0, "/opt/trn_rl_repo")

import concourse.bacc as bacc
import concourse.bass as bass
import concourse.tile as tile
from concourse import mybir
from concourse.bass_utils import run_bass_kernel_spmd

C = 512
CC = C // 128          # 4 chunks of 128
F = 8                  # frames
HW = 1024              # tokens per frame
S = 512                # tokens per half-block / step
NSTEP = 18             # pair-steps per core (balanced)
Q = 1024               # queries per core (two half-blocks)
SW = 64.0              # Wq/Wk host scale
SWV = 16.0             # Wv host scale
SWP = 64.0             # Wp host scale
SU = 64.0              # U quantize scale
SQC = float(np.sqrt(C))
SCALE = 1.0 / SQC
MAGIC = 0x5F3759DF     # Quake rsqrt seed

F32 = mybir.dt.float32
F32R = mybir.dt.float32r
F8 = mybir.dt.float8e4
I32 = mybir.dt.int32
Act = mybir.ActivationFunctionType
Alu = mybir.AluOpType
DR = mybir.MatmulPerfMode.DoubleRow
E4NP = ml_dtypes.float8_e4m3
# column permutation for the stats stream: ss row element p*4+kp holds token
# kp*128+p, so one [1,512]->[128,4] DMA transposes it onto partitions
SSPERM = (np.arange(S) % CC) * 128 + np.arange(S) // CC

_cached = {}


def _build():
    if "nc" in _cached:
        return _cached["nc"]

    nc = bacc.Bacc()
    x8_d = nc.dram_tensor("x8", [128, NSTEP * CC * S], F8, kind="ExternalInput")
    xsq8_d = nc.dram_tensor("xsq8", [128, NSTEP * CC * S], F8, kind="ExternalInput")
    xT8_d = nc.dram_tensor("xT8", [128, NSTEP * CC * C], F8, kind="ExternalInput")
    qoff_d = nc.dram_tensor("qoff", [2, NSTEP], I32, kind="ExternalInput")
    wq8_d = nc.dram_tensor("wq8", [128, CC * C], F8, kind="ExternalInput")
    wk8_d = nc.dram_tensor("wk8", [128, CC * C], F8, kind="ExternalInput")
    wv8_d = nc.dram_tensor("wv8", [128, CC * C], F8, kind="ExternalInput")
    wp8_d = nc.dram_tensor("wp8", [128, CC * C], F8, kind="ExternalInput")
    cvec_d = nc.dram_tensor("cvec", [128, CC], F32, kind="ExternalInput")
    xres_d = nc.dram_tensor("xres", [128, CC * Q], F32, kind="ExternalInput")
    ident_d = nc.dram_tensor("ident", [128, 128], F32, kind="ExternalInput")
    out_d = nc.dram_tensor("out", [C, Q], F32, kind="ExternalOutput")

    with tile.TileContext(nc) as tc:
        with (
            tc.tile_pool(name="const", bufs=1) as const,
            tc.tile_pool(name="persist", bufs=1) as persist,
            tc.tile_pool(name="stream", bufs=4) as stream,
            tc.tile_pool(name="small", bufs=2) as small,
            tc.tile_pool(name="psum_sc", bufs=2, space="PSUM") as psum_sc,
            tc.tile_pool(name="psum_gen", bufs=2, space="PSUM") as psum_gen,
            tc.tile_pool(name="psum_den", bufs=2, space="PSUM") as psum_den,
        ):
            # ---- constant tiles (loads for the q-side path emitted early;
            # wv/wp/xres deferred until after the stream is rolling) ----
            wq8_sb = const.tile([128, CC, C], F8, tag="wq8", name="wq8_sb")
            wk8_sb = const.tile([128, CC, C], F8, tag="wk8", name="wk8_sb")
            wv8_sb = const.tile([128, CC, C], F8, tag="wv8", name="wv8_sb")
            wp8_sb = const.tile([128, CC, C], F8, tag="wp8", name="wp8_sb")
            cvec_sb = const.tile([128, CC], F32, tag="cvec", name="cvec_sb")
            ident_sb = const.tile([128, 128], F32R, tag="ident", name="ident_sb")
            qoff_sb = const.tile([2, NSTEP], I32, tag="qoff", name="qoff_sb")
            xres_sb = const.tile([128, CC, Q], F32, tag="xres", name="xres_sb")
            ones8 = const.tile([128, CC, 16], F8, tag="ones8", name="ones8")
            nc.vector.memset(ones8[:], 1.0)
            invc_sb = const.tile([128, 1], F32, tag="invc", name="invc_sb")
            nc.vector.memset(invc_sb[:], 1.0 / (SWP * SWV * SU))

            def load_early_consts():
                nc.sync.dma_start(out=wq8_sb[:], in_=wq8_d[:])
                nc.sync.dma_start(out=wk8_sb[:], in_=wk8_d[:])
                nc.sync.dma_start(out=cvec_sb[:], in_=cvec_d[:])
                nc.sync.dma_start(out=ident_sb[:], in_=ident_d[:].bitcast(F32R))
                nc.sync.dma_start(out=qoff_sb[:], in_=qoff_d[:])

            def load_late_consts():
                nc.sync.dma_start(out=wv8_sb[:], in_=wv8_d[:])
                nc.sync.dma_start(out=wp8_sb[:], in_=wp8_d[:])
                nc.sync.dma_start(out=xres_sb[:], in_=xres_d[:])

            # ---- persistent accumulators (first pair-step writes them) ----
            qk8_sb = persist.tile([128, CC, Q], F8, tag="qk8", name="qk8_sb")
            U_sb = persist.tile([128, CC, Q], F32, tag="U", name="U_sb")
            den_sb = persist.tile([1, Q], F32, tag="den", name="den_sb")

            # ---- PE warmup: ~4.3us of back-to-back matmuls opens the HAM
            # clock gate (4/8 -> 8/8) before the real stream begins ----
            ones_f = const.tile([128, 1], F32, tag="ones_f", name="ones_f")
            nc.vector.memset(ones_f[:], 1.0)
            ones_r = const.tile([128, 1], F32R, tag="ones_r", name="ones_r")
            nc.vector.tensor_copy(ones_r[:], ones_f[:])
            warm_f = small.tile([128, S], F32, tag="warmf", name="warm_f", bufs=1)
            nc.vector.memset(warm_f[:], 0.0)
            warm_r = small.tile([128, S], F32R, tag="warmr", name="warm_r", bufs=1)
            nc.vector.tensor_copy(warm_r[:], warm_f[:])
            warm_ps = psum_den.tile([1, S], F32, tag="den", name="warm_ps")
            for wi in range(20):
                nc.tensor.matmul(
                    warm_ps[:], ones_r[:], warm_r[:],
                    start=(wi == 0), stop=(wi == 19),
                )

            x8s = {}
            xT8s = {}
            xsq8s = {}
            rqs = {}
            scexps = {}
            lrcs = {}
            inv8s = {}
            # bit-trick log constants: ln(ss) ~= ln2*(bits*2^-23 - 127 + mu)
            C1 = float(-np.log(2.0) / (1 << 24))
            C2 = float(0.5 * np.log(2.0) * (127.0 - 0.0450466) + 0.5 * np.log(C))

            def load_step(i):
                W = CC * S
                xsq8 = stream.tile([128, CC, S], F8, tag="xsq8", name="xsq8", bufs=5)
                nc.sync.dma_start(out=xsq8[:], in_=xsq8_d[:, i * W:(i + 1) * W])
                x8t = stream.tile([128, CC, S], F8, tag="x8", name="x8t", bufs=5)
                nc.sync.dma_start(out=x8t[:], in_=x8_d[:, i * W:(i + 1) * W])
                xT8t = stream.tile([128, CC, C], F8, tag="xT8", name="xT8t", bufs=5)
                nc.sync.dma_start(out=xT8t[:], in_=xT8_d[:, i * W:(i + 1) * W])
                x8s[i] = x8t
                xT8s[i] = xT8t
                xsq8s[i] = xsq8

            def stats_step(i):
                # squares precomputed host-side; channel-sum on PE
                xsq8 = xsq8s[i]
                ssr_ps = psum_den.tile([1, S], F32, tag="den", name="ssr_ps")
                for t in range(2):
                    nc.tensor.matmul(
                        ssr_ps[:],
                        ones8[:, 2 * t:2 * t + 2, 0:1],
                        xsq8[:, 2 * t:2 * t + 2, :],
                        start=(t == 0), stop=(t == 1), perf_mode=DR,
                    )
                ss_row = small.tile([1, S], F32, tag="ssrow", name="ss_row", bufs=3)
                nc.vector.tensor_copy(ss_row[:], ssr_ps[:])
                # xsq8 columns are host-permuted so this single strided DMA
                # lands the per-token row transposed onto partitions
                ssT = small.tile([128, CC], F32, tag="ssT", name="ssT", bufs=3)
                nc.sync.dma_start(out=ssT[:], in_=ss_row[0:1, :])
                # LrC = ln(rho) = ln(sqrt(C)) - 0.5*ln(ss), via exponent-bits log
                bitsf = small.tile([128, CC], F32, tag="bitsf", name="bitsf")
                nc.vector.tensor_copy(bitsf[:], ssT[:].bitcast(I32))
                lrc = stream.tile([128, CC], F32, tag="lrc", name="lrc", bufs=5)
                nc.vector.tensor_scalar(
                    out=lrc[:], in0=bitsf[:],
                    scalar1=C1, scalar2=C2, op0=Alu.mult, op1=Alu.add,
                )
                # Quake rsqrt (1 Newton iter) for the exp scale and den weights
                # (keeps the scalar engine FIFO free for the score exps)
                yi = small.tile([128, CC], I32, tag="qi1", name="yi")
                nc.vector.tensor_scalar(
                    out=yi[:], in0=ssT[:].bitcast(I32),
                    scalar1=1, scalar2=None, op0=Alu.arith_shift_right,
                )
                r0i = small.tile([128, CC], I32, tag="qi2", name="r0i")
                nc.vector.tensor_scalar(
                    out=r0i[:], in0=yi[:],
                    scalar1=-1, scalar2=MAGIC, op0=Alu.mult, op1=Alu.add,
                )
                t1 = small.tile([128, CC], F32, tag="qf1", name="t1")
                nc.vector.tensor_mul(t1[:], ssT[:], r0i[:].bitcast(F32))
                t2 = small.tile([128, CC], F32, tag="qf2", name="t2")
                nc.vector.tensor_mul(t2[:], t1[:], r0i[:].bitcast(F32))
                u = small.tile([128, CC], F32, tag="qf3", name="u")
                nc.vector.tensor_scalar(
                    out=u[:], in0=t2[:],
                    scalar1=-0.5, scalar2=1.5, op0=Alu.mult, op1=Alu.add,
                )
                rq = small.tile([128, CC], F32, tag="qf4", name="rq", bufs=5)
                nc.vector.tensor_mul(rq[:], r0i[:].bitcast(F32), u[:])
                scexp = stream.tile([128, CC], F32, tag="scexp", name="scexp", bufs=5)
                nc.vector.tensor_scalar_mul(scexp[:], rq[:], 1.0 / SW)
                sq = small.tile([128, CC], F32, tag="qf5", name="sq")
                nc.vector.tensor_mul(sq[:], ssT[:], rq[:])
                inv8 = stream.tile([128, CC, 16], F8, tag="inv8", name="inv8", bufs=5)
                nc.vector.tensor_scalar_mul(inv8[:, :, 0:1], sq[:], 1.0 / SQC)
                rqs[i] = rq
                scexps[i] = scexp
                lrcs[i] = lrc
                inv8s[i] = inv8

            def qprep(half, i):
                # rho row for the q tokens: PE mini-transpose of rho cols
                rhoR = small.tile([128, CC], F32R, tag="rhoR", name="rhoR")
                nc.vector.tensor_scalar_mul(rhoR[:], rqs[i][:], SQC)
                row_ps = psum_den.tile([1, S], F32, tag="den", name="row_ps")
                for kp in range(CC):
                    nc.tensor.matmul(
                        row_ps[0:1, kp * 128:(kp + 1) * 128],
                        rhoR[:, kp:kp + 1],
                        ident_sb[:],
                        start=True, stop=True,
                    )
                rho_row = small.tile([1, S], F32, tag="rrow", name="rho_row")
                nc.vector.tensor_scalar_mul(rho_row[:], row_ps[:], 1.0 / SW)
                rho_b = small.tile([128, S], F32, tag="rhob", name="rho_b")
                nc.gpsimd.partition_broadcast(rho_b[:], rho_row[:])
                qn8 = small.tile([128, CC, S], F8, tag="qn8", name="qn8")
                for co in range(CC):
                    q0_ps = psum_gen.tile([128, S], F32, tag="gen", name="q0_ps")
                    for t in range(2):
                        nc.tensor.matmul(
                            q0_ps[:],
                            wq8_sb[:, 2 * t:2 * t + 2, co * 128:(co + 1) * 128],
                            x8s[i][:, 2 * t:2 * t + 2, :],
                            start=(t == 0), stop=(t == 1), perf_mode=DR,
                        )
                    nc.vector.tensor_mul(qn8[:, co, :], q0_ps[:], rho_b[:])
                for ci in range(CC):
                    qk_ps = psum_gen.tile([128, S], F32, tag="gen", name="qk_ps")
                    for t in range(2):
                        nc.tensor.matmul(
                            qk_ps[:],
                            wk8_sb[:, 2 * t:2 * t + 2, ci * 128:(ci + 1) * 128],
                            qn8[:, 2 * t:2 * t + 2, :],
                            start=(t == 0), stop=(t == 1), perf_mode=DR,
                        )
                    nc.vector.tensor_scalar_add(
                        qk8_sb[:, ci, half * S:(half + 1) * S],
                        qk_ps[:], cvec_sb[:, ci:ci + 1],
                    )

            offs = {}
            p8s = {}

            def scores_phase(i):
                off = nc.values_load(
                    qoff_sb[0:1, i:i + 1],
                    engines=[mybir.EngineType.DVE, mybir.EngineType.Pool],
                    min_val=0, max_val=S,
                    skip_runtime_bounds_check=True,
                )
                off4 = nc.values_load(
                    qoff_sb[1:2, i:i + 1],
                    engines=[mybir.EngineType.DVE],
                    min_val=0, max_val=S // 4,
                    skip_runtime_bounds_check=True,
                )
                offs[i] = off
                # the dual-fp8 ISA check rejects register offsets on the
                # matmul rhs, so materialize this step's q-half of qk with a
                # DVE copy (register offsets are fine there)
                qkc = stream.tile([128, CC, S], F8, tag="qkc", name="qkc", bufs=3)
                nc.vector.tensor_copy(qkc[:], qk8_sb[:, :, bass.ds(off, S)])
                p8t = stream.tile([128, CC, S], F8, tag="p8", name="p8t", bufs=3)
                for kh in range(2):
                    s_ps = psum_sc.tile([128, 2, S], F32, tag="sc", name="s_ps")
                    for kp2 in range(2):
                        kp = kh * 2 + kp2
                        for t in range(2):
                            nc.tensor.matmul(
                                s_ps[:, kp2, :],
                                x8s[i][:, 2 * t:2 * t + 2, kp * 128:(kp + 1) * 128],
                                qkc[:, 2 * t:2 * t + 2, :],
                                start=(t == 0), stop=(t == 1), perf_mode=DR,
                            )
                        nc.scalar.activation(
                            p8t[:, kp, :], s_ps[:, kp2, :], Act.Exp,
                            bias=lrcs[i][:, kp:kp + 1],
                            scale=scexps[i][:, kp:kp + 1],
                        )
                p8s[i] = p8t

            def accum_phase(i):
                off = offs.pop(i)
                p8t = p8s.pop(i)
                first = i < 2  # steps 0/1 are the first touch of their q-half
                dn_ps = psum_den.tile([1, S], F32, tag="den", name="dn_ps")
                for t in range(2):
                    nc.tensor.matmul(
                        dn_ps[:],
                        inv8s[i][:, 2 * t:2 * t + 2, 0:1],
                        p8t[:, 2 * t:2 * t + 2, :],
                        start=(t == 0), stop=(t == 1), perf_mode=DR,
                    )
                if first:
                    nc.vector.tensor_copy(den_sb[:, bass.ds(off, S)], dn_ps[:])
                else:
                    nc.vector.tensor_add(
                        den_sb[:, bass.ds(off, S)], den_sb[:, bass.ds(off, S)],
                        dn_ps[:],
                    )
                for ci in range(CC):
                    u_ps = psum_gen.tile([128, S], F32, tag="gen", name="u_ps")
                    for t in range(2):
                        nc.tensor.matmul(
                            u_ps[:],
                            xT8s[i][:, 2 * t:2 * t + 2, ci * 128:(ci + 1) * 128],
                            p8t[:, 2 * t:2 * t + 2, :],
                            start=(t == 0), stop=(t == 1), perf_mode=DR,
                        )
                    if first:
                        nc.vector.tensor_copy(U_sb[:, ci, bass.ds(off, S)], u_ps[:])
                    else:
                        nc.vector.tensor_add(
                            U_sb[:, ci, bass.ds(off, S)],
                            U_sb[:, ci, bass.ds(off, S)],
                            u_ps[:],
                        )

            u8s = {}

            def fin_pre(half):
                cols = half * S
                dent = small.tile([1, S], F32, tag="rrow", name="dent")
                nc.vector.tensor_scalar_mul(dent[:], den_sb[:, cols:cols + S], 1.0 / SU)
                denb = small.tile([128, S], F32, tag="denb", name="denb")
                nc.gpsimd.partition_broadcast(denb[:], dent[:])
                rdb = small.tile([128, S], F32, tag="rhob", name="rdb")
                nc.vector.reciprocal(rdb[:], denb[:])
                u8 = small.tile([128, CC, S], F8, tag="u8", name="u8")
                for ci in range(CC):
                    nc.vector.tensor_mul(u8[:, ci, :], U_sb[:, ci, cols:cols + S], rdb[:])
                u8s[half] = u8

            def fin_proj(half):
                cols = half * S
                u8 = u8s[half]
                o8 = small.tile([128, CC, S], F8, tag="o8", name="o8")
                for co in range(CC):
                    ot_ps = psum_gen.tile([128, S], F32, tag="gen", name="ot_ps")
                    for t in range(2):
                        nc.tensor.matmul(
                            ot_ps[:],
                            wv8_sb[:, 2 * t:2 * t + 2, co * 128:(co + 1) * 128],
                            u8[:, 2 * t:2 * t + 2, :],
                            start=(t == 0), stop=(t == 1), perf_mode=DR,
                        )
                    nc.vector.tensor_copy(o8[:, co, :], ot_ps[:])
                for co in range(CC):
                    pr_ps = psum_gen.tile([128, S], F32, tag="gen", name="pr_ps")
                    for t in range(2):
                        nc.tensor.matmul(
                            pr_ps[:],
                            wp8_sb[:, 2 * t:2 * t + 2, co * 128:(co + 1) * 128],
                            o8[:, 2 * t:2 * t + 2, :],
                            start=(t == 0), stop=(t == 1), perf_mode=DR,
                        )
                    res = small.tile([128, S], F32, tag="res", name="res")
                    nc.vector.scalar_tensor_tensor(
                        out=res[:],
                        in0=pr_ps[:],
                        scalar=invc_sb[:],
                        in1=xres_sb[:, co, cols:cols + S],
                        op0=Alu.mult,
                        op1=Alu.add,
                    )
                    nc.sync.dma_start(
                        out=out_d[co * 128:(co + 1) * 128, cols:cols + S], in_=res[:],
                    )

            # ---- schedule: stream loads first, then q-prep, then the
            # software-pipelined pair steps (scores of i+1 overlap the
            # exp/accumulate of i) ----
            PF = 3
            load_step(0)
            load_step(1)
            load_early_consts()
            load_step(2)
            stats_step(0)
            stats_step(1)
            stats_step(2)
            qprep(0, 0)
            qprep(1, 1)
            scores_phase(0)
            for i in range(NSTEP):
                if i + PF < NSTEP:
                    load_step(i + PF)
                    stats_step(i + PF)
                if i == 0:
                    load_late_consts()
                if i + 1 < NSTEP:
                    scores_phase(i + 1)
                accum_phase(i)
            fin_pre(0)
            fin_pre(1)
            fin_proj(0)
            fin_proj(1)

    nc.finalize()
    _cached["nc"] = nc
    return nc


def _q8(a):
    a = np.clip(np.asarray(a, np.float32), -240.0, 240.0)
    return a.astype(E4NP)


def _prep_inputs(x, gamma, wq, bq, wk, bk, wv, bv, wp, bp):
    x = np.asarray(x, np.float32)
    X = np.ascontiguousarray(x[0].reshape(C, F * HW))
    g = np.asarray(gamma, np.float32)
    wq = np.asarray(wq, np.float32)
    wk = np.asarray(wk, np.float32)
    wv = np.asarray(wv, np.float32)
    wp = np.asarray(wp, np.float32)
    bq = np.asarray(bq, np.float32)
    bv = np.asarray(bv, np.float32)
    bp = np.asarray(bp, np.float32)

    def pack_cols(a):
        # [C, n] -> [128, CC*n]: row p, col (ci, j) = a[ci*128+p, j]
        n = a.shape[1]
        return np.ascontiguousarray(
            a.reshape(CC, 128, n).transpose(1, 0, 2).reshape(128, CC * n)
        )

    wq8 = pack_cols(_q8(SW * (wq * g[None, :]).T))      # [cin, o]
    wk8 = pack_cols(_q8(SW * (wk * g[None, :])))        # [o, cin]
    wv8 = pack_cols(_q8(SWV * (wv * g[None, :]).T))     # [cin, o']
    wp8 = pack_cols(_q8(SWP * wp.T))                    # [o', co]
    cvec = (SW * (wk * g[None, :]).T @ bq).astype(np.float32)
    cvec_p = np.ascontiguousarray(cvec.reshape(CC, 128).T)
    bvp = (bp + wp @ bv).astype(np.float32)

    X8 = _q8(X)                              # [C, seq] fp8
    XSQ8 = _q8(X8.astype(np.float32) ** 2)   # squares of the quantized x
    XT8 = np.ascontiguousarray(X8.T)         # [seq, C] fp8
    ident = np.eye(128, dtype=np.float32)

    common = {
        "wq8": wq8, "wk8": wk8, "wv8": wv8, "wp8": wp8,
        "cvec": cvec_p,
        "ident": ident,
    }
    in_maps = []
    for j in range(F):
        p, h = j // 2, j % 2
        fa, fb = p, F - 1 - p
        ba, bb = 2 * fa + h, 2 * fb + h
        steps = [ba, bb]
        steps += [b for b in range(2 * fa + 2) if b != ba]
        steps += [b for b in range(2 * fb + 2) if b != bb]
        assert len(steps) == NSTEP
        qoffs = [0, S] + [0] * (2 * fa + 1) + [S] * (2 * fb + 1)
        m = dict(common)
        # packed per-step tiles: [128, NSTEP*CC*S]
        m["x8"] = np.concatenate(
            [pack_cols(X8[:, b * S:(b + 1) * S]) for b in steps], axis=1
        )
        m["xsq8"] = np.concatenate(
            [pack_cols(XSQ8[:, b * S:(b + 1) * S][:, SSPERM]) for b in steps],
            axis=1,
        )
        # xT8 tile layout: row p, col (kp, c) = XT8[b*S + kp*128 + p, c]
        m["xT8"] = np.concatenate(
            [XT8[b * S:(b + 1) * S, :].reshape(CC, 128, C)
             .transpose(1, 0, 2).reshape(128, CC * C) for b in steps],
            axis=1,
        )
        m["xT8"] = np.ascontiguousarray(m["xT8"])
        m["qoff"] = np.asarray(
            [qoffs, [q // 4 for q in qoffs]], np.int32)
        xres = np.concatenate(
            [X[:, ba * S:(ba + 1) * S], X[:, bb * S:(bb + 1) * S]], axis=1
        ) + bvp[:, None]
        m["xres"] = pack_cols(xres.astype(np.float32))
        in_maps.append(m)
    return in_maps


def kernel(x, gamma, wq, bq, wk, bk, wv, bv, wp, bp, _trace=False):
    nc = _build()
    in_maps = _prep_inputs(x, gamma, wq, bq, wk, bk, wv, bv, wp, bp)
    kwargs = {}
    if _trace:
        kwargs = dict(trace=True, trace_cores=list(range(F)))
    r = run_bass_kernel_spmd(nc, in_maps, core_ids=list(range(F)), **kwargs)
    out = np.empty((1, C, F, HW), np.float32)
    for j in range(F):
        p, h = j // 2, j % 2
        fa, fb = p, F - 1 - p
        res = r.results[j]["out"]
        out[0, :, fa, h * S:h * S + S] = res[:, 0:S]
        out[0, :, fb, h * S:h * S + S] = res[:, S:Q]
    out = out.reshape(1, C, F, 32, 32)
    kernel._last_results = r
    return out


# revision 30
# speedup vs baseline: 1.1739x; 1.1739x over previous
"""Block-causal attention block (RMSnorm + QKV + frame-causal attention + proj)
on 8 TRN2 NeuronCores — fp8 DoubleRow + weight-folding edition.

Sharding: as the baseline — core j (p=j//2, h=j%2) owns query half-blocks
(frame p, col-half h) and (frame 7-p, col-half h); each core runs a uniform
stream of 18 (kv half-block, q-half) pair-steps (perfectly balanced since
2(p+1) + 2(8-p) = 18).

Algebraic folds (vs the baseline's per-step K/V projections):
  - K-fold: scores s[kv,q] = x_raw[:,kv]^T (Wk_fold q) — the K projection
    moves to the query side (one GEMM per q-half); the k-bias term is
    constant per query row and drops by softmax shift invariance.
  - V-fold: O = Wv_fold (sum_kv xn[:,kv] p[kv,q]) — the V projection moves
    to after the attention sum (one GEMM per q-half); bv folds through wp
    into the output bias since softmax rows sum to 1.
  - Norm-fold: RMS norm scalar rho[t] = sqrt(C)/||x_t|| is computed on the
    DVE (free-axis square-reduce of the transposed slab + Quake rsqrt) and
    applied as (a) the per-partition scale AP of the Exp activation on the
    kv side and (b) a broadcast row multiply on the q side. gamma folds
    into the weights host-side.

All matmuls run in fp8 e4m3 with perf_mode=DoubleRow (K=256 per
instruction, 2 fp8 MACs/cell/cycle). Weights are pre-scaled by 64 (16 for
Wv) host-side to sit in e4m3's normal range; the compensations fold into
the exp scale and two output-side constants. Residual x + bias stays f32.

Per-pair-step PE work: 8 DR matmuls (scores) + 8 (U accum) + 2 (den)
vs the baseline's 64+8 full-rate f32r matmuls per step.
"""

import sys

import numpy as np
import ml_dtypes

sys.path.insert(0, "/opt/trn_rl_repo")

import concourse.bacc as bacc
import concourse.bass as bass
import concourse.tile as tile
from concourse import mybir
from concourse.bass_utils import run_bass_kernel_spmd

C = 512
CC = C // 128          # 4 chunks of 128
F = 8                  # frames
HW = 1024              # tokens per frame
S = 512                # tokens per half-block / step
NSTEP = 18             # pair-steps per core (balanced)
Q = 1024               # queries per core (two half-blocks)
SW = 64.0              # Wq/Wk host scale
SWV = 16.0             # Wv host scale
SWP = 64.0             # Wp host scale
SU = 64.0              # U quantize scale
SQC = float(np.sqrt(C))
SCALE = 1.0 / SQC
MAGIC = 0x5F3759DF     # Quake rsqrt seed

F32 = mybir.dt.float32
F32R = mybir.dt.float32r
F8 = mybir.dt.float8e4
I32 = mybir.dt.int32
Act = mybir.ActivationFunctionType
Alu = mybir.AluOpType
DR = mybir.MatmulPerfMode.DoubleRow
E4NP = ml_dtypes.float8_e4m3
# column permutation for the stats stream: ss row element p*4+kp holds token
# kp*128+p, so one [1,512]->[128,4] DMA transposes it onto partitions
SSPERM = (np.arange(S) % CC) * 128 + np.arange(S) // CC

_cached = {}


def _build():
    if "nc" in _cached:
        return _cached["nc"]

    nc = bacc.Bacc()
    x8_d = nc.dram_tensor("x8", [128, NSTEP * CC * S], F8, kind="ExternalInput")
    xsq8_d = nc.dram_tensor("xsq8", [128, NSTEP * CC * S], F8, kind="ExternalInput")
    xT8_d = nc.dram_tensor("xT8", [128, NSTEP * CC * C], F8, kind="ExternalInput")
    qoff_d = nc.dram_tensor("qoff", [1, NSTEP], I32, kind="ExternalInput")
    wq8_d = nc.dram_tensor("wq8", [128, CC * C], F8, kind="ExternalInput")
    wk8_d = nc.dram_tensor("wk8", [128, CC * C], F8, kind="ExternalInput")
    wv8_d = nc.dram_tensor("wv8", [128, CC * C], F8, kind="ExternalInput")
    wp8_d = nc.dram_tensor("wp8", [128, CC * C], F8, kind="ExternalInput")
    cvec_d = nc.dram_tensor("cvec", [128, CC], F32, kind="ExternalInput")
    xres_d = nc.dram_tensor("xres", [128, CC * Q], F32, kind="ExternalInput")
    ident_d = nc.dram_tensor("ident", [128, 128], F32, kind="ExternalInput")
    out_d = nc.dram_tensor("out", [C, Q], F32, kind="ExternalOutput")

    with tile.TileContext(nc) as tc:
        with (
            tc.tile_pool(name="const", bufs=1) as const,
            tc.tile_pool(name="persist", bufs=1) as persist,
            tc.tile_pool(name="stream", bufs=4) as stream,
            tc.tile_pool(name="small", bufs=2) as small,
            tc.tile_pool(name="psum_sc", bufs=2, space="PSUM") as psum_sc,
            tc.tile_pool(name="psum_gen", bufs=2, space="PSUM") as psum_gen,
            tc.tile_pool(name="psum_den", bufs=2, space="PSUM") as psum_den,
        ):
            # ---- constant tiles (loads for the q-side path emitted early;
            # wv/wp/xres deferred until after the stream is rolling) ----
            wq8_sb = const.tile([128, CC, C], F8, tag="wq8", name="wq8_sb")
            wk8_sb = const.tile([128, CC, C], F8, tag="wk8", name="wk8_sb")
            wv8_sb = const.tile([128, CC, C], F8, tag="wv8", name="wv8_sb")
            wp8_sb = const.tile([128, CC, C], F8, tag="wp8", name="wp8_sb")
            cvec_sb = const.tile([128, CC], F32, tag="cvec", name="cvec_sb")
            ident_sb = const.tile([128, 128], F32R, tag="ident", name="ident_sb")
            qoff_sb = const.tile([1, NSTEP], I32, tag="qoff", name="qoff_sb")
            xres_sb = const.tile([128, CC, Q], F32, tag="xres", name="xres_sb")
            ones8 = const.tile([128, CC, 16], F8, tag="ones8", name="ones8")
            nc.vector.memset(ones8[:], 1.0)
            invc_sb = const.tile([128, 1], F32, tag="invc", name="invc_sb")
            nc.vector.memset(invc_sb[:], 1.0 / (SWP * SWV * SU))

            def load_early_consts():
                nc.sync.dma_start(out=wq8_sb[:], in_=wq8_d[:])
                nc.sync.dma_start(out=wk8_sb[:], in_=wk8_d[:])
                nc.sync.dma_start(out=cvec_sb[:], in_=cvec_d[:])
                nc.sync.dma_start(out=ident_sb[:], in_=ident_d[:].bitcast(F32R))
                nc.sync.dma_start(out=qoff_sb[:], in_=qoff_d[:])

            def load_late_consts():
                nc.sync.dma_start(out=wv8_sb[:], in_=wv8_d[:])
                nc.sync.dma_start(out=wp8_sb[:], in_=wp8_d[:])
                nc.sync.dma_start(out=xres_sb[:], in_=xres_d[:])

            # ---- persistent accumulators (first pair-step writes them) ----
            qk8_sb = persist.tile([128, CC, Q], F8, tag="qk8", name="qk8_sb")
            U_sb = persist.tile([128, CC, Q], F32, tag="U", name="U_sb")
            den_sb = persist.tile([1, Q], F32, tag="den", name="den_sb")

            # ---- PE warmup: ~4.3us of back-to-back matmuls opens the HAM
            # clock gate (4/8 -> 8/8) before the real stream begins ----
            ones_f = const.tile([128, 1], F32, tag="ones_f", name="ones_f")
            nc.vector.memset(ones_f[:], 1.0)
            ones_r = const.tile([128, 1], F32R, tag="ones_r", name="ones_r")
            nc.vector.tensor_copy(ones_r[:], ones_f[:])
            warm_f = small.tile([128, S], F32, tag="warmf", name="warm_f", bufs=1)
            nc.vector.memset(warm_f[:], 0.0)
            warm_r = small.tile([128, S], F32R, tag="warmr", name="warm_r", bufs=1)
            nc.vector.tensor_copy(warm_r[:], warm_f[:])
            warm_ps = psum_den.tile([1, S], F32, tag="den", name="warm_ps")
            for wi in range(20):
                nc.tensor.matmul(
                    warm_ps[:], ones_r[:], warm_r[:],
                    start=(wi == 0), stop=(wi == 19),
                )

            x8s = {}
            xT8s = {}
            xsq8s = {}
            rqs = {}
            scexps = {}
            lrcs = {}
            inv8s = {}
            # bit-trick log constants: ln(ss) ~= ln2*(bits*2^-23 - 127 + mu)
            C1 = float(-np.log(2.0) / (1 << 24))
            C2 = float(0.5 * np.log(2.0) * (127.0 - 0.0450466) + 0.5 * np.log(C))

            def load_step(i):
                W = CC * S
                xsq8 = stream.tile([128, CC, S], F8, tag="xsq8", name="xsq8", bufs=5)
                nc.sync.dma_start(out=xsq8[:], in_=xsq8_d[:, i * W:(i + 1) * W])
                x8t = stream.tile([128, CC, S], F8, tag="x8", name="x8t", bufs=5)
                nc.sync.dma_start(out=x8t[:], in_=x8_d[:, i * W:(i + 1) * W])
                xT8t = stream.tile([128, CC, C], F8, tag="xT8", name="xT8t", bufs=5)
                nc.sync.dma_start(out=xT8t[:], in_=xT8_d[:, i * W:(i + 1) * W])
                x8s[i] = x8t
                xT8s[i] = xT8t
                xsq8s[i] = xsq8

            def stats_step(i):
                # squares precomputed host-side; channel-sum on PE
                xsq8 = xsq8s[i]
                ssr_ps = psum_den.tile([1, S], F32, tag="den", name="ssr_ps")
                for t in range(2):
                    nc.tensor.matmul(
                        ssr_ps[:],
                        ones8[:, 2 * t:2 * t + 2, 0:1],
                        xsq8[:, 2 * t:2 * t + 2, :],
                        start=(t == 0), stop=(t == 1), perf_mode=DR,
                    )
                ss_row = small.tile([1, S], F32, tag="ssrow", name="ss_row", bufs=3)
                nc.vector.tensor_copy(ss_row[:], ssr_ps[:])
                # xsq8 columns are host-permuted so this single strided DMA
                # lands the per-token row transposed onto partitions
                ssT = small.tile([128, CC], F32, tag="ssT", name="ssT", bufs=3)
                nc.sync.dma_start(out=ssT[:], in_=ss_row[0:1, :])
                # LrC = ln(rho) = ln(sqrt(C)) - 0.5*ln(ss), via exponent-bits log
                bitsf = small.tile([128, CC], F32, tag="bitsf", name="bitsf")
                nc.vector.tensor_copy(bitsf[:], ssT[:].bitcast(I32))
                lrc = stream.tile([128, CC], F32, tag="lrc", name="lrc", bufs=5)
                nc.vector.tensor_scalar(
                    out=lrc[:], in0=bitsf[:],
                    scalar1=C1, scalar2=C2, op0=Alu.mult, op1=Alu.add,
                )
                # Quake rsqrt (1 Newton iter) for the exp scale and den weights
                # (keeps the scalar engine FIFO free for the score exps)
                yi = small.tile([128, CC], I32, tag="qi1", name="yi")
                nc.vector.tensor_scalar(
                    out=yi[:], in0=ssT[:].bitcast(I32),
                    scalar1=1, scalar2=None, op0=Alu.arith_shift_right,
                )
                r0i = small.tile([128, CC], I32, tag="qi2", name="r0i")
                nc.vector.tensor_scalar(
                    out=r0i[:], in0=yi[:],
                    scalar1=-1, scalar2=MAGIC, op0=Alu.mult, op1=Alu.add,
                )
                t1 = small.tile([128, CC], F32, tag="qf1", name="t1")
                nc.vector.tensor_mul(t1[:], ssT[:], r0i[:].bitcast(F32))
                t2 = small.tile([128, CC], F32, tag="qf2", name="t2")
                nc.vector.tensor_mul(t2[:], t1[:], r0i[:].bitcast(F32))
                u = small.tile([128, CC], F32, tag="qf3", name="u")
                nc.vector.tensor_scalar(
                    out=u[:], in0=t2[:],
                    scalar1=-0.5, scalar2=1.5, op0=Alu.mult, op1=Alu.add,
                )
                rq = small.tile([128, CC], F32, tag="qf4", name="rq", bufs=5)
                nc.vector.tensor_mul(rq[:], r0i[:].bitcast(F32), u[:])
                scexp = stream.tile([128, CC], F32, tag="scexp", name="scexp", bufs=5)
                nc.vector.tensor_scalar_mul(scexp[:], rq[:], 1.0 / SW)
                sq = small.tile([128, CC], F32, tag="qf5", name="sq")
                nc.vector.tensor_mul(sq[:], ssT[:], rq[:])
                inv8 = stream.tile([128, CC, 16], F8, tag="inv8", name="inv8", bufs=5)
                nc.vector.tensor_scalar_mul(inv8[:, :, 0:1], sq[:], 1.0 / SQC)
                rqs[i] = rq
                scexps[i] = scexp
                lrcs[i] = lrc
                inv8s[i] = inv8

            def qprep(half, i):
                # rho row for the q tokens: PE mini-transpose of rho cols
                rhoR = small.tile([128, CC], F32R, tag="rhoR", name="rhoR")
                nc.vector.tensor_scalar_mul(rhoR[:], rqs[i][:], SQC)
                row_ps = psum_den.tile([1, S], F32, tag="den", name="row_ps")
                for kp in range(CC):
                    nc.tensor.matmul(
                        row_ps[0:1, kp * 128:(kp + 1) * 128],
                        rhoR[:, kp:kp + 1],
                        ident_sb[:],
                        start=True, stop=True,
                    )
                rho_row = small.tile([1, S], F32, tag="rrow", name="rho_row")
                nc.vector.tensor_scalar_mul(rho_row[:], row_ps[:], 1.0 / SW)
                rho_b = small.tile([128, S], F32, tag="rhob", name="rho_b")
                nc.gpsimd.partition_broadcast(rho_b[:], rho_row[:])
                qn8 = small.tile([128, CC, S], F8, tag="qn8", name="qn8")
                for co in range(CC):
                    q0_ps = psum_gen.tile([128, S], F32, tag="gen", name="q0_ps")
                    for t in range(2):
                        nc.tensor.matmul(
                            q0_ps[:],
                            wq8_sb[:, 2 * t:2 * t + 2, co * 128:(co + 1) * 128],
                            x8s[i][:, 2 * t:2 * t + 2, :],
                            start=(t == 0), stop=(t == 1), perf_mode=DR,
                        )
                    nc.vector.tensor_mul(qn8[:, co, :], q0_ps[:], rho_b[:])
                for ci in range(CC):
                    qk_ps = psum_gen.tile([128, S], F32, tag="gen", name="qk_ps")
                    for t in range(2):
                        nc.tensor.matmul(
                            qk_ps[:],
                            wk8_sb[:, 2 * t:2 * t + 2, ci * 128:(ci + 1) * 128],
                            qn8[:, 2 * t:2 * t + 2, :],
                            start=(t == 0), stop=(t == 1), perf_mode=DR,
                        )
                    nc.vector.tensor_scalar_add(
                        qk8_sb[:, ci, half * S:(half + 1) * S],
                        qk_ps[:], cvec_sb[:, ci:ci + 1],
                    )

            offs = {}
            p8s = {}

            def scores_phase(i):
                off = nc.values_load(
                    qoff_sb[0:1, i:i + 1],
                    engines=[mybir.EngineType.DVE],
                    min_val=0, max_val=S,
                    skip_runtime_bounds_check=True,
                )
                offs[i] = off
                # the dual-fp8 ISA check rejects register offsets on the
                # matmul rhs, so materialize this step's q-half of qk with a
                # DVE copy (register offsets are fine there)
                qkc = stream.tile([128, CC, S], F8, tag="qkc", name="qkc", bufs=3)
                nc.vector.tensor_copy(qkc[:], qk8_sb[:, :, bass.ds(off, S)])
                p8t = stream.tile([128, CC, S], F8, tag="p8", name="p8t", bufs=3)
                for kh in range(2):
                    s_ps = psum_sc.tile([128, 2, S], F32, tag="sc", name="s_ps")
                    for kp2 in range(2):
                        kp = kh * 2 + kp2
                        for t in range(2):
                            nc.tensor.matmul(
                                s_ps[:, kp2, :],
                                x8s[i][:, 2 * t:2 * t + 2, kp * 128:(kp + 1) * 128],
                                qkc[:, 2 * t:2 * t + 2, :],
                                start=(t == 0), stop=(t == 1), perf_mode=DR,
                            )
                        nc.scalar.activation(
                            p8t[:, kp, :], s_ps[:, kp2, :], Act.Exp,
                            bias=lrcs[i][:, kp:kp + 1],
                            scale=scexps[i][:, kp:kp + 1],
                        )
                p8s[i] = p8t

            def accum_phase(i):
                off = offs.pop(i)
                p8t = p8s.pop(i)
                first = i < 2  # steps 0/1 are the first touch of their q-half
                dn_ps = psum_den.tile([1, S], F32, tag="den", name="dn_ps")
                for t in range(2):
                    nc.tensor.matmul(
                        dn_ps[:],
                        inv8s[i][:, 2 * t:2 * t + 2, 0:1],
                        p8t[:, 2 * t:2 * t + 2, :],
                        start=(t == 0), stop=(t == 1), perf_mode=DR,
                    )
                if first:
                    nc.vector.tensor_copy(den_sb[:, bass.ds(off, S)], dn_ps[:])
                else:
                    nc.vector.tensor_add(
                        den_sb[:, bass.ds(off, S)], den_sb[:, bass.ds(off, S)],
                        dn_ps[:],
                    )
                for ci in range(CC):
                    u_ps = psum_gen.tile([128, S], F32, tag="gen", name="u_ps")
                    for t in range(2):
                        nc.tensor.matmul(
                            u_ps[:],
                            xT8s[i][:, 2 * t:2 * t + 2, ci * 128:(ci + 1) * 128],
                            p8t[:, 2 * t:2 * t + 2, :],
                            start=(t == 0), stop=(t == 1), perf_mode=DR,
                        )
                    if first:
                        nc.vector.tensor_copy(U_sb[:, ci, bass.ds(off, S)], u_ps[:])
                    else:
                        nc.vector.tensor_add(
                            U_sb[:, ci, bass.ds(off, S)],
                            U_sb[:, ci, bass.ds(off, S)],
                            u_ps[:],
                        )

            u8s = {}

            def fin_pre(half):
                cols = half * S
                dent = small.tile([1, S], F32, tag="rrow", name="dent")
                nc.vector.tensor_scalar_mul(dent[:], den_sb[:, cols:cols + S], 1.0 / SU)
                denb = small.tile([128, S], F32, tag="denb", name="denb")
                nc.gpsimd.partition_broadcast(denb[:], dent[:])
                rdb = small.tile([128, S], F32, tag="rhob", name="rdb")
                nc.vector.reciprocal(rdb[:], denb[:])
                u8 = small.tile([128, CC, S], F8, tag="u8", name="u8")
                for ci in range(CC):
                    nc.vector.tensor_mul(u8[:, ci, :], U_sb[:, ci, cols:cols + S], rdb[:])
                u8s[half] = u8

            def fin_proj(half):
                cols = half * S
                u8 = u8s[half]
                o8 = small.tile([128, CC, S], F8, tag="o8", name="o8")
                for co in range(CC):
                    ot_ps = psum_gen.tile([128, S], F32, tag="gen", name="ot_ps")
                    for t in range(2):
                        nc.tensor.matmul(
                            ot_ps[:],
                            wv8_sb[:, 2 * t:2 * t + 2, co * 128:(co + 1) * 128],
                            u8[:, 2 * t:2 * t + 2, :],
                            start=(t == 0), stop=(t == 1), perf_mode=DR,
                        )
                    nc.vector.tensor_copy(o8[:, co, :], ot_ps[:])
                for co in range(CC):
                    pr_ps = psum_gen.tile([128, S], F32, tag="gen", name="pr_ps")
                    for t in range(2):
                        nc.tensor.matmul(
                            pr_ps[:],
                            wp8_sb[:, 2 * t:2 * t + 2, co * 128:(co + 1) * 128],
                            o8[:, 2 * t:2 * t + 2, :],
                            start=(t == 0), stop=(t == 1), perf_mode=DR,
                        )
                    res = small.tile([128, S], F32, tag="res", name="res")
                    nc.vector.scalar_tensor_tensor(
                        out=res[:],
                        in0=pr_ps[:],
                        scalar=invc_sb[:],
                        in1=xres_sb[:, co, cols:cols + S],
                        op0=Alu.mult,
                        op1=Alu.add,
                    )
                    nc.sync.dma_start(
                        out=out_d[co * 128:(co + 1) * 128, cols:cols + S], in_=res[:],
                    )

            # ---- schedule: stream loads first, then q-prep, then the
            # software-pipelined pair steps (scores of i+1 overlap the
            # exp/accumulate of i) ----
            PF = 3
            load_step(0)
            load_step(1)
            load_early_consts()
            load_step(2)
            stats_step(0)
            stats_step(1)
            stats_step(2)
            qprep(0, 0)
            qprep(1, 1)
            scores_phase(0)
            for i in range(NSTEP):
                if i + PF < NSTEP:
                    load_step(i + PF)
                    stats_step(i + PF)
                if i == 0:
                    load_late_consts()
                if i + 1 < NSTEP:
                    scores_phase(i + 1)
                accum_phase(i)
            fin_pre(0)
            fin_pre(1)
            fin_proj(0)
            fin_proj(1)

    nc.finalize()
    _cached["nc"] = nc
    return nc


def _q8(a):
    a = np.clip(np.asarray(a, np.float32), -240.0, 240.0)
    return a.astype(E4NP)


def _prep_inputs(x, gamma, wq, bq, wk, bk, wv, bv, wp, bp):
    x = np.asarray(x, np.float32)
    X = np.ascontiguousarray(x[0].reshape(C, F * HW))
    g = np.asarray(gamma, np.float32)
    wq = np.asarray(wq, np.float32)
    wk = np.asarray(wk, np.float32)
    wv = np.asarray(wv, np.float32)
    wp = np.asarray(wp, np.float32)
    bq = np.asarray(bq, np.float32)
    bv = np.asarray(bv, np.float32)
    bp = np.asarray(bp, np.float32)

    def pack_cols(a):
        # [C, n] -> [128, CC*n]: row p, col (ci, j) = a[ci*128+p, j]
        n = a.shape[1]
        return np.ascontiguousarray(
            a.reshape(CC, 128, n).transpose(1, 0, 2).reshape(128, CC * n)
        )

    wq8 = pack_cols(_q8(SW * (wq * g[None, :]).T))      # [cin, o]
    wk8 = pack_cols(_q8(SW * (wk * g[None, :])))        # [o, cin]
    wv8 = pack_cols(_q8(SWV * (wv * g[None, :]).T))     # [cin, o']
    wp8 = pack_cols(_q8(SWP * wp.T))                    # [o', co]
    cvec = (SW * (wk * g[None, :]).T @ bq).astype(np.float32)
    cvec_p = np.ascontiguousarray(cvec.reshape(CC, 128).T)
    bvp = (bp + wp @ bv).astype(np.float32)

    X8 = _q8(X)                              # [C, seq] fp8
    XSQ8 = _q8(X8.astype(np.float32) ** 2)   # squares of the quantized x
    XT8 = np.ascontiguousarray(X8.T)         # [seq, C] fp8
    ident = np.eye(128, dtype=np.float32)

    common = {
        "wq8": wq8, "wk8": wk8, "wv8": wv8, "wp8": wp8,
        "cvec": cvec_p,
        "ident": ident,
    }
    in_maps = []
    for j in range(F):
        p, h = j // 2, j % 2
        fa, fb = p, F - 1 - p
        ba, bb = 2 * fa + h, 2 * fb + h
        steps = [ba, bb]
        steps += [b for b in range(2 * fa + 2) if b != ba]
        steps += [b for b in range(2 * fb + 2) if b != bb]
        assert len(steps) == NSTEP
        qoffs = [0, S] + [0] * (2 * fa + 1) + [S] * (2 * fb + 1)
        m = dict(common)
        # packed per-step tiles: [128, NSTEP*CC*S]
        m["x8"] = np.concatenate(
            [pack_cols(X8[:, b * S:(b + 1) * S]) for b in steps], axis=1
        )
        m["xsq8"] = np.concatenate(
            [pack_cols(XSQ8[:, b * S:(b + 1) * S][:, SSPERM]) for b in steps],
            axis=1,
        )
        # xT8 tile layout: row p, col (kp, c) = XT8[b*S + kp*128 + p, c]
        m["xT8"] = np.concatenate(
            [XT8[b * S:(b + 1) * S, :].reshape(CC, 128, C)
             .transpose(1, 0, 2).reshape(128, CC * C) for b in steps],
            axis=1,
        )
        m["xT8"] = np.ascontiguousarray(m["xT8"])
        m["qoff"] = np.asarray([qoffs], np.int32)
        xres = np.concatenate(
            [X[:, ba * S:(ba + 1) * S], X[:, bb * S:(bb + 1) * S]], axis=1
        ) + bvp[:, None]
        m["xres"] = pack_cols(xres.astype(np.float32))
        in_maps.append(m)
    return in_maps


def kernel(x, gamma, wq, bq, wk, bk, wv, bv, wp, bp, _trace=False):
    nc = _build()
    in_maps = _prep_inputs(x, gamma, wq, bq, wk, bk, wv, bv, wp, bp)
    kwargs = {}
    if _trace:
        kwargs = dict(trace=True, trace_cores=list(range(F)))
    r = run_bass_kernel_spmd(nc, in_maps, core_ids=list(range(F)), **kwargs)
    out = np.empty((1, C, F, HW), np.float32)
    for j in range(F):
        p, h = j // 2, j % 2
        fa, fb = p, F - 1 - p
        res = r.results[j]["out"]
        out[0, :, fa, h * S:h * S + S] = res[:, 0:S]
        out[0, :, fb, h * S:h * S + S] = res[:, S:Q]
    out = out.reshape(1, C, F, 32, 32)
    kernel._last_results = r
    return out


# revision 31
# speedup vs baseline: 1.2473x; 1.0625x over previous
"""Block-causal attention block (RMSnorm + QKV + frame-causal attention + proj)
on 8 TRN2 NeuronCores — fp8 DoubleRow + weight-folding edition.

Sharding: as the baseline — core j (p=j//2, h=j%2) owns query half-blocks
(frame p, col-half h) and (frame 7-p, col-half h); each core runs a uniform
stream of 18 (kv half-block, q-half) pair-steps (perfectly balanced since
2(p+1) + 2(8-p) = 18).

Algebraic folds (vs the baseline's per-step K/V projections):
  - K-fold: scores s[kv,q] = x_raw[:,kv]^T (Wk_fold q) — the K projection
    moves to the query side (one GEMM per q-half); the k-bias term is
    constant per query row and drops by softmax shift invariance.
  - V-fold: O = Wv_fold (sum_kv xn[:,kv] p[kv,q]) — the V projection moves
    to after the attention sum (one GEMM per q-half); bv folds through wp
    into the output bias since softmax rows sum to 1.
  - Norm-fold: RMS norm scalar rho[t] = sqrt(C)/||x_t|| is computed on the
    DVE (free-axis square-reduce of the transposed slab + Quake rsqrt) and
    applied as (a) the per-partition scale AP of the Exp activation on the
    kv side and (b) a broadcast row multiply on the q side. gamma folds
    into the weights host-side.

All matmuls run in fp8 e4m3 with perf_mode=DoubleRow (K=256 per
instruction, 2 fp8 MACs/cell/cycle). Weights are pre-scaled by 64 (16 for
Wv) host-side to sit in e4m3's normal range; the compensations fold into
the exp scale and two output-side constants. Residual x + bias stays f32.

Per-pair-step PE work: 8 DR matmuls (scores) + 8 (U accum) + 2 (den)
vs the baseline's 64+8 full-rate f32r matmuls per step.
"""

import sys

import numpy as np
import ml_dtypes

sys.path.insert(0, "/opt/trn_rl_repo")

import concourse.bacc as bacc
import concourse.bass as bass
import concourse.tile as tile
from concourse import mybir
from concourse.bass_utils import run_bass_kernel_spmd

C = 512
CC = C // 128          # 4 chunks of 128
F = 8                  # frames
HW = 1024              # tokens per frame
S = 512                # tokens per half-block / step
NSTEP = 18             # pair-steps per core (balanced)
Q = 1024               # queries per core (two half-blocks)
SW = 64.0              # Wq/Wk host scale
SWV = 16.0             # Wv host scale
SWP = 64.0             # Wp host scale
SU = 64.0              # U quantize scale
SQC = float(np.sqrt(C))
SCALE = 1.0 / SQC
MAGIC = 0x5F3759DF     # Quake rsqrt seed

F32 = mybir.dt.float32
F32R = mybir.dt.float32r
F8 = mybir.dt.float8e4
I32 = mybir.dt.int32
Act = mybir.ActivationFunctionType
Alu = mybir.AluOpType
DR = mybir.MatmulPerfMode.DoubleRow
E4NP = ml_dtypes.float8_e4m3
# column permutation for the stats stream: ss row element p*4+kp holds token
# kp*128+p, so one [1,512]->[128,4] DMA transposes it onto partitions
SSPERM = (np.arange(S) % CC) * 128 + np.arange(S) // CC

_cached = {}


def _build():
    if "nc" in _cached:
        return _cached["nc"]

    nc = bacc.Bacc()
    x8_d = nc.dram_tensor("x8", [128, NSTEP * CC * S], F8, kind="ExternalInput")
    xsq8_d = nc.dram_tensor("xsq8", [128, NSTEP * CC * S], F8, kind="ExternalInput")
    xT8_d = nc.dram_tensor("xT8", [128, NSTEP * CC * C], F8, kind="ExternalInput")
    qoff_d = nc.dram_tensor("qoff", [1, NSTEP], I32, kind="ExternalInput")
    wq8_d = nc.dram_tensor("wq8", [128, CC * C], F8, kind="ExternalInput")
    wk8_d = nc.dram_tensor("wk8", [128, CC * C], F8, kind="ExternalInput")
    wv8_d = nc.dram_tensor("wv8", [128, CC * C], F8, kind="ExternalInput")
    wp8_d = nc.dram_tensor("wp8", [128, CC * C], F8, kind="ExternalInput")
    cvec_d = nc.dram_tensor("cvec", [128, CC], F32, kind="ExternalInput")
    xres_d = nc.dram_tensor("xres", [128, CC * Q], F32, kind="ExternalInput")
    ident_d = nc.dram_tensor("ident", [128, 128], F32, kind="ExternalInput")
    out_d = nc.dram_tensor("out", [C, Q], F32, kind="ExternalOutput")

    with tile.TileContext(nc) as tc:
        with (
            tc.tile_pool(name="const", bufs=1) as const,
            tc.tile_pool(name="persist", bufs=1) as persist,
            tc.tile_pool(name="stream", bufs=4) as stream,
            tc.tile_pool(name="small", bufs=2) as small,
            tc.tile_pool(name="psum_sc", bufs=2, space="PSUM") as psum_sc,
            tc.tile_pool(name="psum_gen", bufs=3, space="PSUM") as psum_gen,
            tc.tile_pool(name="psum_den", bufs=2, space="PSUM") as psum_den,
        ):
            # ---- constant tiles (loads for the q-side path emitted early;
            # wv/wp/xres deferred until after the stream is rolling) ----
            wq8_sb = const.tile([128, CC, C], F8, tag="wq8", name="wq8_sb")
            wk8_sb = const.tile([128, CC, C], F8, tag="wk8", name="wk8_sb")
            wv8_sb = const.tile([128, CC, C], F8, tag="wv8", name="wv8_sb")
            wp8_sb = const.tile([128, CC, C], F8, tag="wp8", name="wp8_sb")
            cvec_sb = const.tile([128, CC], F32, tag="cvec", name="cvec_sb")
            ident_sb = const.tile([128, 128], F32R, tag="ident", name="ident_sb")
            qoff_sb = const.tile([1, NSTEP], I32, tag="qoff", name="qoff_sb")
            xres_sb = const.tile([128, CC, Q], F32, tag="xres", name="xres_sb")
            ones8 = const.tile([128, CC, 16], F8, tag="ones8", name="ones8")
            nc.vector.memset(ones8[:], 1.0)
            invc_sb = const.tile([128, 1], F32, tag="invc", name="invc_sb")
            nc.vector.memset(invc_sb[:], 1.0 / (SWP * SWV * SU))

            def load_early_consts():
                nc.sync.dma_start(out=wq8_sb[:], in_=wq8_d[:])
                nc.sync.dma_start(out=wk8_sb[:], in_=wk8_d[:])
                nc.sync.dma_start(out=cvec_sb[:], in_=cvec_d[:])
                nc.sync.dma_start(out=ident_sb[:], in_=ident_d[:].bitcast(F32R))
                nc.sync.dma_start(out=qoff_sb[:], in_=qoff_d[:])

            def load_late_consts():
                nc.sync.dma_start(out=wv8_sb[:], in_=wv8_d[:])
                nc.sync.dma_start(out=wp8_sb[:], in_=wp8_d[:])
                nc.sync.dma_start(out=xres_sb[:], in_=xres_d[:])

            # ---- persistent accumulators (first pair-step writes them) ----
            qk8_sb = persist.tile([128, CC, Q], F8, tag="qk8", name="qk8_sb")
            U_sb = persist.tile([128, CC, Q], F32, tag="U", name="U_sb")
            den_sb = persist.tile([1, Q], F32, tag="den", name="den_sb")

            # ---- PE warmup: ~4.3us of back-to-back matmuls opens the HAM
            # clock gate (4/8 -> 8/8) before the real stream begins ----
            ones_f = const.tile([128, 1], F32, tag="ones_f", name="ones_f")
            nc.vector.memset(ones_f[:], 1.0)
            ones_r = const.tile([128, 1], F32R, tag="ones_r", name="ones_r")
            nc.vector.tensor_copy(ones_r[:], ones_f[:])
            warm_f = small.tile([128, S], F32, tag="warmf", name="warm_f", bufs=1)
            nc.vector.memset(warm_f[:], 0.0)
            warm_r = small.tile([128, S], F32R, tag="warmr", name="warm_r", bufs=1)
            nc.vector.tensor_copy(warm_r[:], warm_f[:])
            warm_ps = psum_den.tile([1, S], F32, tag="den", name="warm_ps")
            for wi in range(20):
                nc.tensor.matmul(
                    warm_ps[:], ones_r[:], warm_r[:],
                    start=(wi == 0), stop=(wi == 19),
                )

            x8s = {}
            xT8s = {}
            xsq8s = {}
            rqs = {}
            scexps = {}
            lrcs = {}
            inv8s = {}
            # bit-trick log constants: ln(ss) ~= ln2*(bits*2^-23 - 127 + mu)
            C1 = float(-np.log(2.0) / (1 << 24))
            C2 = float(0.5 * np.log(2.0) * (127.0 - 0.0450466) + 0.5 * np.log(C))

            def load_step(i):
                W = CC * S
                xsq8 = stream.tile([128, CC, S], F8, tag="xsq8", name="xsq8", bufs=5)
                nc.sync.dma_start(out=xsq8[:], in_=xsq8_d[:, i * W:(i + 1) * W])
                x8t = stream.tile([128, CC, S], F8, tag="x8", name="x8t", bufs=5)
                nc.sync.dma_start(out=x8t[:], in_=x8_d[:, i * W:(i + 1) * W])
                xT8t = stream.tile([128, CC, C], F8, tag="xT8", name="xT8t", bufs=5)
                nc.sync.dma_start(out=xT8t[:], in_=xT8_d[:, i * W:(i + 1) * W])
                x8s[i] = x8t
                xT8s[i] = xT8t
                xsq8s[i] = xsq8

            def stats_step(i):
                # squares precomputed host-side; channel-sum on PE
                xsq8 = xsq8s[i]
                ssr_ps = psum_den.tile([1, S], F32, tag="den", name="ssr_ps")
                for t in range(2):
                    nc.tensor.matmul(
                        ssr_ps[:],
                        ones8[:, 2 * t:2 * t + 2, 0:1],
                        xsq8[:, 2 * t:2 * t + 2, :],
                        start=(t == 0), stop=(t == 1), perf_mode=DR,
                    )
                ss_row = small.tile([1, S], F32, tag="ssrow", name="ss_row", bufs=3)
                nc.vector.tensor_copy(ss_row[:], ssr_ps[:])
                # xsq8 columns are host-permuted so this single strided DMA
                # lands the per-token row transposed onto partitions
                ssT = small.tile([128, CC], F32, tag="ssT", name="ssT", bufs=3)
                nc.sync.dma_start(out=ssT[:], in_=ss_row[0:1, :])
                # LrC = ln(rho) = ln(sqrt(C)) - 0.5*ln(ss), via exponent-bits log
                bitsf = small.tile([128, CC], F32, tag="bitsf", name="bitsf")
                nc.vector.tensor_copy(bitsf[:], ssT[:].bitcast(I32))
                lrc = stream.tile([128, CC], F32, tag="lrc", name="lrc", bufs=5)
                nc.vector.tensor_scalar(
                    out=lrc[:], in0=bitsf[:],
                    scalar1=C1, scalar2=C2, op0=Alu.mult, op1=Alu.add,
                )
                # Quake rsqrt (1 Newton iter) for the exp scale and den weights
                # (keeps the scalar engine FIFO free for the score exps)
                yi = small.tile([128, CC], I32, tag="qi1", name="yi")
                nc.vector.tensor_scalar(
                    out=yi[:], in0=ssT[:].bitcast(I32),
                    scalar1=1, scalar2=None, op0=Alu.arith_shift_right,
                )
                r0i = small.tile([128, CC], I32, tag="qi2", name="r0i")
                nc.vector.tensor_scalar(
                    out=r0i[:], in0=yi[:],
                    scalar1=-1, scalar2=MAGIC, op0=Alu.mult, op1=Alu.add,
                )
                t1 = small.tile([128, CC], F32, tag="qf1", name="t1")
                nc.vector.tensor_mul(t1[:], ssT[:], r0i[:].bitcast(F32))
                t2 = small.tile([128, CC], F32, tag="qf2", name="t2")
                nc.vector.tensor_mul(t2[:], t1[:], r0i[:].bitcast(F32))
                u = small.tile([128, CC], F32, tag="qf3", name="u")
                nc.vector.tensor_scalar(
                    out=u[:], in0=t2[:],
                    scalar1=-0.5, scalar2=1.5, op0=Alu.mult, op1=Alu.add,
                )
                rq = small.tile([128, CC], F32, tag="qf4", name="rq", bufs=5)
                nc.vector.tensor_mul(rq[:], r0i[:].bitcast(F32), u[:])
                scexp = stream.tile([128, CC], F32, tag="scexp", name="scexp", bufs=5)
                nc.vector.tensor_scalar_mul(scexp[:], rq[:], 1.0 / SW)
                sq = small.tile([128, CC], F32, tag="qf5", name="sq")
                nc.vector.tensor_mul(sq[:], ssT[:], rq[:])
                inv8 = stream.tile([128, CC, 16], F8, tag="inv8", name="inv8", bufs=5)
                nc.vector.tensor_scalar_mul(inv8[:, :, 0:1], sq[:], 1.0 / SQC)
                rqs[i] = rq
                scexps[i] = scexp
                lrcs[i] = lrc
                inv8s[i] = inv8

            def qprep(half, i):
                # rho row for the q tokens: PE mini-transpose of rho cols
                rhoR = small.tile([128, CC], F32R, tag="rhoR", name="rhoR")
                nc.vector.tensor_scalar_mul(rhoR[:], rqs[i][:], SQC)
                row_ps = psum_den.tile([1, S], F32, tag="den", name="row_ps")
                for kp in range(CC):
                    nc.tensor.matmul(
                        row_ps[0:1, kp * 128:(kp + 1) * 128],
                        rhoR[:, kp:kp + 1],
                        ident_sb[:],
                        start=True, stop=True,
                    )
                rho_row = small.tile([1, S], F32, tag="rrow", name="rho_row")
                nc.vector.tensor_scalar_mul(rho_row[:], row_ps[:], 1.0 / SW)
                rho_b = small.tile([128, S], F32, tag="rhob", name="rho_b")
                nc.gpsimd.partition_broadcast(rho_b[:], rho_row[:])
                qn8 = small.tile([128, CC, S], F8, tag="qn8", name="qn8")
                for co in range(CC):
                    q0_ps = psum_gen.tile([128, S], F32, tag="gen", name="q0_ps")
                    for t in range(2):
                        nc.tensor.matmul(
                            q0_ps[:],
                            wq8_sb[:, 2 * t:2 * t + 2, co * 128:(co + 1) * 128],
                            x8s[i][:, 2 * t:2 * t + 2, :],
                            start=(t == 0), stop=(t == 1), perf_mode=DR,
                        )
                    nc.vector.tensor_mul(qn8[:, co, :], q0_ps[:], rho_b[:])
                for ci in range(CC):
                    qk_ps = psum_gen.tile([128, S], F32, tag="gen", name="qk_ps")
                    for t in range(2):
                        nc.tensor.matmul(
                            qk_ps[:],
                            wk8_sb[:, 2 * t:2 * t + 2, ci * 128:(ci + 1) * 128],
                            qn8[:, 2 * t:2 * t + 2, :],
                            start=(t == 0), stop=(t == 1), perf_mode=DR,
                        )
                    nc.vector.tensor_scalar_add(
                        qk8_sb[:, ci, half * S:(half + 1) * S],
                        qk_ps[:], cvec_sb[:, ci:ci + 1],
                    )

            offs = {}
            p8s = {}
            qkcs = {}

            def prep_pair(i):
                off = nc.values_load(
                    qoff_sb[0:1, i:i + 1],
                    engines=[mybir.EngineType.DVE],
                    min_val=0, max_val=S,
                    skip_runtime_bounds_check=True,
                )
                offs[i] = off
                # the dual-fp8 ISA check rejects register offsets on the
                # matmul rhs, so materialize this step's q-half of qk with a
                # DVE copy (register offsets are fine there)
                qkc = stream.tile([128, CC, S], F8, tag="qkc", name="qkc", bufs=4)
                nc.vector.tensor_copy(qkc[:], qk8_sb[:, :, bass.ds(off, S)])
                qkcs[i] = qkc

            def scores_phase(i):
                qkc = qkcs.pop(i)
                p8t = stream.tile([128, CC, S], F8, tag="p8", name="p8t", bufs=3)
                for kp in range(CC):
                    s_ps = psum_sc.tile([128, S], F32, tag="sc", name="s_ps", bufs=3)
                    for t in range(2):
                        nc.tensor.matmul(
                            s_ps[:],
                            x8s[i][:, 2 * t:2 * t + 2, kp * 128:(kp + 1) * 128],
                            qkc[:, 2 * t:2 * t + 2, :],
                            start=(t == 0), stop=(t == 1), perf_mode=DR,
                        )
                    nc.scalar.activation(
                        p8t[:, kp, :], s_ps[:], Act.Exp,
                        bias=lrcs[i][:, kp:kp + 1],
                        scale=scexps[i][:, kp:kp + 1],
                    )
                p8s[i] = p8t

            def accum_phase(i):
                off = offs.pop(i)
                p8t = p8s.pop(i)
                first = i < 2  # steps 0/1 are the first touch of their q-half
                dn_ps = psum_den.tile([1, S], F32, tag="den", name="dn_ps")
                for t in range(2):
                    nc.tensor.matmul(
                        dn_ps[:],
                        inv8s[i][:, 2 * t:2 * t + 2, 0:1],
                        p8t[:, 2 * t:2 * t + 2, :],
                        start=(t == 0), stop=(t == 1), perf_mode=DR,
                    )
                if first:
                    nc.vector.tensor_copy(den_sb[:, bass.ds(off, S)], dn_ps[:])
                else:
                    nc.vector.tensor_add(
                        den_sb[:, bass.ds(off, S)], den_sb[:, bass.ds(off, S)],
                        dn_ps[:],
                    )
                for ci in range(CC):
                    u_ps = psum_gen.tile([128, S], F32, tag="gen", name="u_ps")
                    for t in range(2):
                        nc.tensor.matmul(
                            u_ps[:],
                            xT8s[i][:, 2 * t:2 * t + 2, ci * 128:(ci + 1) * 128],
                            p8t[:, 2 * t:2 * t + 2, :],
                            start=(t == 0), stop=(t == 1), perf_mode=DR,
                        )
                    if first:
                        nc.vector.tensor_copy(U_sb[:, ci, bass.ds(off, S)], u_ps[:])
                    else:
                        nc.vector.tensor_add(
                            U_sb[:, ci, bass.ds(off, S)],
                            U_sb[:, ci, bass.ds(off, S)],
                            u_ps[:],
                        )

            u8s = {}

            def fin_pre(half):
                cols = half * S
                dent = small.tile([1, S], F32, tag="rrow", name="dent")
                nc.vector.tensor_scalar_mul(dent[:], den_sb[:, cols:cols + S], 1.0 / SU)
                denb = small.tile([128, S], F32, tag="denb", name="denb")
                nc.gpsimd.partition_broadcast(denb[:], dent[:])
                rdb = small.tile([128, S], F32, tag="rhob", name="rdb")
                nc.vector.reciprocal(rdb[:], denb[:])
                u8 = small.tile([128, CC, S], F8, tag="u8", name="u8")
                for ci in range(CC):
                    nc.vector.tensor_mul(u8[:, ci, :], U_sb[:, ci, cols:cols + S], rdb[:])
                u8s[half] = u8

            def fin_proj(half):
                cols = half * S
                u8 = u8s[half]
                o8 = small.tile([128, CC, S], F8, tag="o8", name="o8")
                for co in range(CC):
                    ot_ps = psum_gen.tile([128, S], F32, tag="gen", name="ot_ps")
                    for t in range(2):
                        nc.tensor.matmul(
                            ot_ps[:],
                            wv8_sb[:, 2 * t:2 * t + 2, co * 128:(co + 1) * 128],
                            u8[:, 2 * t:2 * t + 2, :],
                            start=(t == 0), stop=(t == 1), perf_mode=DR,
                        )
                    nc.vector.tensor_copy(o8[:, co, :], ot_ps[:])
                for co in range(CC):
                    pr_ps = psum_gen.tile([128, S], F32, tag="gen", name="pr_ps")
                    for t in range(2):
                        nc.tensor.matmul(
                            pr_ps[:],
                            wp8_sb[:, 2 * t:2 * t + 2, co * 128:(co + 1) * 128],
                            o8[:, 2 * t:2 * t + 2, :],
                            start=(t == 0), stop=(t == 1), perf_mode=DR,
                        )
                    res = small.tile([128, S], F32, tag="res", name="res")
                    nc.vector.scalar_tensor_tensor(
                        out=res[:],
                        in0=pr_ps[:],
                        scalar=invc_sb[:],
                        in1=xres_sb[:, co, cols:cols + S],
                        op0=Alu.mult,
                        op1=Alu.add,
                    )
                    nc.sync.dma_start(
                        out=out_d[co * 128:(co + 1) * 128, cols:cols + S], in_=res[:],
                    )

            # ---- schedule: stream loads first, then q-prep, then the
            # software-pipelined pair steps (scores of i+1 overlap the
            # exp/accumulate of i) ----
            PF = 3
            load_step(0)
            load_step(1)
            load_early_consts()
            load_step(2)
            stats_step(0)
            stats_step(1)
            stats_step(2)
            qprep(0, 0)
            qprep(1, 1)
            prep_pair(0)
            prep_pair(1)
            scores_phase(0)
            for i in range(NSTEP):
                if i + PF < NSTEP:
                    load_step(i + PF)
                    stats_step(i + PF)
                if i == 0:
                    load_late_consts()
                if i + 2 < NSTEP:
                    prep_pair(i + 2)
                if i + 1 < NSTEP:
                    scores_phase(i + 1)
                accum_phase(i)
            fin_pre(0)
            fin_pre(1)
            fin_proj(0)
            fin_proj(1)

    nc.finalize()
    _cached["nc"] = nc
    return nc


def _q8(a):
    a = np.clip(np.asarray(a, np.float32), -240.0, 240.0)
    return a.astype(E4NP)


def _prep_inputs(x, gamma, wq, bq, wk, bk, wv, bv, wp, bp):
    x = np.asarray(x, np.float32)
    X = np.ascontiguousarray(x[0].reshape(C, F * HW))
    g = np.asarray(gamma, np.float32)
    wq = np.asarray(wq, np.float32)
    wk = np.asarray(wk, np.float32)
    wv = np.asarray(wv, np.float32)
    wp = np.asarray(wp, np.float32)
    bq = np.asarray(bq, np.float32)
    bv = np.asarray(bv, np.float32)
    bp = np.asarray(bp, np.float32)

    def pack_cols(a):
        # [C, n] -> [128, CC*n]: row p, col (ci, j) = a[ci*128+p, j]
        n = a.shape[1]
        return np.ascontiguousarray(
            a.reshape(CC, 128, n).transpose(1, 0, 2).reshape(128, CC * n)
        )

    wq8 = pack_cols(_q8(SW * (wq * g[None, :]).T))      # [cin, o]
    wk8 = pack_cols(_q8(SW * (wk * g[None, :])))        # [o, cin]
    wv8 = pack_cols(_q8(SWV * (wv * g[None, :]).T))     # [cin, o']
    wp8 = pack_cols(_q8(SWP * wp.T))                    # [o', co]
    cvec = (SW * (wk * g[None, :]).T @ bq).astype(np.float32)
    cvec_p = np.ascontiguousarray(cvec.reshape(CC, 128).T)
    bvp = (bp + wp @ bv).astype(np.float32)

    X8 = _q8(X)                              # [C, seq] fp8
    XSQ8 = _q8(X8.astype(np.float32) ** 2)   # squares of the quantized x
    XT8 = np.ascontiguousarray(X8.T)         # [seq, C] fp8
    ident = np.eye(128, dtype=np.float32)

    common = {
        "wq8": wq8, "wk8": wk8, "wv8": wv8, "wp8": wp8,
        "cvec": cvec_p,
        "ident": ident,
    }
    in_maps = []
    for j in range(F):
        p, h = j // 2, j % 2
        fa, fb = p, F - 1 - p
        ba, bb = 2 * fa + h, 2 * fb + h
        steps = [ba, bb]
        steps += [b for b in range(2 * fa + 2) if b != ba]
        steps += [b for b in range(2 * fb + 2) if b != bb]
        assert len(steps) == NSTEP
        qoffs = [0, S] + [0] * (2 * fa + 1) + [S] * (2 * fb + 1)
        m = dict(common)
        # packed per-step tiles: [128, NSTEP*CC*S]
        m["x8"] = np.concatenate(
            [pack_cols(X8[:, b * S:(b + 1) * S]) for b in steps], axis=1
        )
        m["xsq8"] = np.concatenate(
            [pack_cols(XSQ8[:, b * S:(b + 1) * S][:, SSPERM]) for b in steps],
            axis=1,
        )
        # xT8 tile layout: row p, col (kp, c) = XT8[b*S + kp*128 + p, c]
        m["xT8"] = np.concatenate(
            [XT8[b * S:(b + 1) * S, :].reshape(CC, 128, C)
             .transpose(1, 0, 2).reshape(128, CC * C) for b in steps],
            axis=1,
        )
        m["xT8"] = np.ascontiguousarray(m["xT8"])
        m["qoff"] = np.asarray([qoffs], np.int32)
        xres = np.concatenate(
            [X[:, ba * S:(ba + 1) * S], X[:, bb * S:(bb + 1) * S]], axis=1
        ) + bvp[:, None]
        m["xres"] = pack_cols(xres.astype(np.float32))
        in_maps.append(m)
    return in_maps


def kernel(x, gamma, wq, bq, wk, bk, wv, bv, wp, bp, _trace=False):
    nc = _build()
    in_maps = _prep_inputs(x, gamma, wq, bq, wk, bk, wv, bv, wp, bp)
    kwargs = {}
    if _trace:
        kwargs = dict(trace=True, trace_cores=list(range(F)))
    r = run_bass_kernel_spmd(nc, in_maps, core_ids=list(range(F)), **kwargs)
    out = np.empty((1, C, F, HW), np.float32)
    for j in range(F):
        p, h = j // 2, j % 2
        fa, fb = p, F - 1 - p
        res = r.results[j]["out"]
        out[0, :, fa, h * S:h * S + S] = res[:, 0:S]
        out[0, :, fb, h * S:h * S + S] = res[:, S:Q]
    out = out.reshape(1, C, F, 32, 32)
    kernel._last_results = r
    return out


# revision 33
# speedup vs baseline: 1.4373x; 1.1523x over previous
"""Block-causal attention block (RMSnorm + QKV + frame-causal attention + proj)
on 8 TRN2 NeuronCores — fp8 DoubleRow + weight-folding edition.

Sharding: as the baseline — core j (p=j//2, h=j%2) owns query half-blocks
(frame p, col-half h) and (frame 7-p, col-half h); each core runs a uniform
stream of 18 (kv half-block, q-half) pair-steps (perfectly balanced since
2(p+1) + 2(8-p) = 18).

Algebraic folds (vs the baseline's per-step K/V projections):
  - K-fold: scores s[kv,q] = x_raw[:,kv]^T (Wk_fold q) — the K projection
    moves to the query side (one GEMM per q-half); the k-bias term is
    constant per query row and drops by softmax shift invariance.
  - V-fold: O = Wv_fold (sum_kv xn[:,kv] p[kv,q]) — the V projection moves
    to after the attention sum (one GEMM per q-half); bv folds through wp
    into the output bias since softmax rows sum to 1.
  - Norm-fold: RMS norm scalar rho[t] = sqrt(C)/||x_t|| is computed on the
    DVE (free-axis square-reduce of the transposed slab + Quake rsqrt) and
    applied as (a) the per-partition scale AP of the Exp activation on the
    kv side and (b) a broadcast row multiply on the q side. gamma folds
    into the weights host-side.

All matmuls run in fp8 e4m3 with perf_mode=DoubleRow (K=256 per
instruction, 2 fp8 MACs/cell/cycle). Weights are pre-scaled by 64 (16 for
Wv) host-side to sit in e4m3's normal range; the compensations fold into
the exp scale and two output-side constants. Residual x + bias stays f32.

Per-pair-step PE work: 8 DR matmuls (scores) + 8 (U accum) + 2 (den)
vs the baseline's 64+8 full-rate f32r matmuls per step.
"""

import sys

import numpy as np
import ml_dtypes

sys.path.insert(0, "/opt/trn_rl_repo")

import concourse.bacc as bacc
import concourse.bass as bass
import concourse.tile as tile
from concourse import mybir
from concourse.bass_utils import run_bass_kernel_spmd

C = 512
CC = C // 128          # 4 chunks of 128
F = 8                  # frames
HW = 1024              # tokens per frame
S = 512                # tokens per half-block / step
NSTEP = 18             # pair-steps per core (balanced)
Q = 1024               # queries per core (two half-blocks)
SW = 64.0              # Wq/Wk host scale
SWVP = 256.0           # fused Wv*Wp host scale
SU = 64.0              # U quantize scale
SQC = float(np.sqrt(C))
SCALE = 1.0 / SQC
MAGIC = 0x5F3759DF     # Quake rsqrt seed

F32 = mybir.dt.float32
F32R = mybir.dt.float32r
F8 = mybir.dt.float8e4
I32 = mybir.dt.int32
Act = mybir.ActivationFunctionType
Alu = mybir.AluOpType
DR = mybir.MatmulPerfMode.DoubleRow
E4NP = ml_dtypes.float8_e4m3
# column permutation for the stats stream: ss row element p*4+kp holds token
# kp*128+p, so one [1,512]->[128,4] DMA transposes it onto partitions
SSPERM = (np.arange(S) % CC) * 128 + np.arange(S) // CC

_cached = {}


def _build():
    if "nc" in _cached:
        return _cached["nc"]

    nc = bacc.Bacc()
    x8_d = nc.dram_tensor("x8", [128, NSTEP * CC * S], F8, kind="ExternalInput")
    xsq8_d = nc.dram_tensor("xsq8", [128, NSTEP * CC * S], F8, kind="ExternalInput")
    xT8_d = nc.dram_tensor("xT8", [128, NSTEP * CC * C], F8, kind="ExternalInput")
    qoff_d = nc.dram_tensor("qoff", [1, NSTEP], I32, kind="ExternalInput")
    wq8_d = nc.dram_tensor("wq8", [128, CC * C], F8, kind="ExternalInput")
    wk8_d = nc.dram_tensor("wk8", [128, CC * C], F8, kind="ExternalInput")
    wvp8_d = nc.dram_tensor("wvp8", [128, CC * C], F8, kind="ExternalInput")
    cvec_d = nc.dram_tensor("cvec", [128, CC], F32, kind="ExternalInput")
    xres_d = nc.dram_tensor("xres", [128, CC * Q], F32, kind="ExternalInput")
    ident_d = nc.dram_tensor("ident", [128, 128], F32, kind="ExternalInput")
    out_d = nc.dram_tensor("out", [C, Q], F32, kind="ExternalOutput")

    with tile.TileContext(nc) as tc:
        with (
            tc.tile_pool(name="const", bufs=1) as const,
            tc.tile_pool(name="persist", bufs=1) as persist,
            tc.tile_pool(name="stream", bufs=4) as stream,
            tc.tile_pool(name="small", bufs=2) as small,
            tc.tile_pool(name="psum_sc", bufs=2, space="PSUM") as psum_sc,
            tc.tile_pool(name="psum_gen", bufs=3, space="PSUM") as psum_gen,
            tc.tile_pool(name="psum_den", bufs=2, space="PSUM") as psum_den,
        ):
            # ---- constant tiles (loads for the q-side path emitted early;
            # wv/wp/xres deferred until after the stream is rolling) ----
            wq8_sb = const.tile([128, CC, C], F8, tag="wq8", name="wq8_sb")
            wk8_sb = const.tile([128, CC, C], F8, tag="wk8", name="wk8_sb")
            wvp8_sb = const.tile([128, CC, C], F8, tag="wvp8", name="wvp8_sb")
            cvec_sb = const.tile([128, CC], F32, tag="cvec", name="cvec_sb")
            ident_sb = const.tile([128, 128], F32R, tag="ident", name="ident_sb")
            qoff_sb = const.tile([1, NSTEP], I32, tag="qoff", name="qoff_sb")
            xres_sb = const.tile([128, CC, Q], F32, tag="xres", name="xres_sb")
            ones8 = const.tile([128, CC, 16], F8, tag="ones8", name="ones8")
            nc.vector.memset(ones8[:], 1.0)
            invc_sb = const.tile([128, 1], F32, tag="invc", name="invc_sb")
            nc.vector.memset(invc_sb[:], 1.0 / (SWVP * SU))

            def load_early_consts():
                nc.sync.dma_start(out=wq8_sb[:], in_=wq8_d[:])
                nc.sync.dma_start(out=wk8_sb[:], in_=wk8_d[:])
                nc.sync.dma_start(out=cvec_sb[:], in_=cvec_d[:])
                nc.sync.dma_start(out=ident_sb[:], in_=ident_d[:].bitcast(F32R))
                nc.sync.dma_start(out=qoff_sb[:], in_=qoff_d[:])

            def load_late_consts():
                nc.sync.dma_start(out=wvp8_sb[:], in_=wvp8_d[:])
                nc.sync.dma_start(out=xres_sb[:], in_=xres_d[:])

            # ---- persistent accumulators (first pair-step writes them) ----
            qk8_sb = persist.tile([128, CC, Q], F8, tag="qk8", name="qk8_sb")
            U_sb = persist.tile([128, CC, Q], F32, tag="U", name="U_sb")
            den_sb = persist.tile([1, Q], F32, tag="den", name="den_sb")

            # ---- PE warmup: ~4.3us of back-to-back matmuls opens the HAM
            # clock gate (4/8 -> 8/8) before the real stream begins ----
            ones_f = const.tile([128, 1], F32, tag="ones_f", name="ones_f")
            nc.vector.memset(ones_f[:], 1.0)
            ones_r = const.tile([128, 1], F32R, tag="ones_r", name="ones_r")
            nc.vector.tensor_copy(ones_r[:], ones_f[:])
            warm_f = small.tile([128, S], F32, tag="warmf", name="warm_f", bufs=1)
            nc.vector.memset(warm_f[:], 0.0)
            warm_r = small.tile([128, S], F32R, tag="warmr", name="warm_r", bufs=1)
            nc.vector.tensor_copy(warm_r[:], warm_f[:])
            warm_ps = psum_den.tile([1, S], F32, tag="den", name="warm_ps")
            for wi in range(20):
                nc.tensor.matmul(
                    warm_ps[:], ones_r[:], warm_r[:],
                    start=(wi == 0), stop=(wi == 19),
                )

            x8s = {}
            xT8s = {}
            xsq8s = {}
            rqs = {}
            scexps = {}
            lrcs = {}
            inv8s = {}
            # bit-trick log constants: ln(ss) ~= ln2*(bits*2^-23 - 127 + mu)
            C1 = float(-np.log(2.0) / (1 << 24))
            C2 = float(0.5 * np.log(2.0) * (127.0 - 0.0450466) + 0.5 * np.log(C))

            def load_step(i):
                W = CC * S
                xsq8 = stream.tile([128, CC, S], F8, tag="xsq8", name="xsq8", bufs=7)
                nc.sync.dma_start(out=xsq8[:], in_=xsq8_d[:, i * W:(i + 1) * W])
                x8t = stream.tile([128, CC, S], F8, tag="x8", name="x8t", bufs=7)
                nc.sync.dma_start(out=x8t[:], in_=x8_d[:, i * W:(i + 1) * W])
                xT8t = stream.tile([128, CC, C], F8, tag="xT8", name="xT8t", bufs=7)
                nc.sync.dma_start(out=xT8t[:], in_=xT8_d[:, i * W:(i + 1) * W])
                x8s[i] = x8t
                xT8s[i] = xT8t
                xsq8s[i] = xsq8

            def stats_step(i):
                # squares precomputed host-side; channel-sum on PE
                xsq8 = xsq8s[i]
                ssr_ps = psum_den.tile([1, S], F32, tag="den", name="ssr_ps")
                for t in range(2):
                    nc.tensor.matmul(
                        ssr_ps[:],
                        ones8[:, 2 * t:2 * t + 2, 0:1],
                        xsq8[:, 2 * t:2 * t + 2, :],
                        start=(t == 0), stop=(t == 1), perf_mode=DR,
                    )
                ss_row = small.tile([1, S], F32, tag="ssrow", name="ss_row", bufs=3)
                nc.vector.tensor_copy(ss_row[:], ssr_ps[:])
                # xsq8 columns are host-permuted so this single strided DMA
                # lands the per-token row transposed onto partitions
                ssT = small.tile([128, CC], F32, tag="ssT", name="ssT", bufs=3)
                nc.sync.dma_start(out=ssT[:], in_=ss_row[0:1, :])
                # LrC = ln(rho) = ln(sqrt(C)) - 0.5*ln(ss), via exponent-bits log
                bitsf = small.tile([128, CC], F32, tag="bitsf", name="bitsf")
                nc.vector.tensor_copy(bitsf[:], ssT[:].bitcast(I32))
                lrc = stream.tile([128, CC], F32, tag="lrc", name="lrc", bufs=7)
                nc.vector.tensor_scalar(
                    out=lrc[:], in0=bitsf[:],
                    scalar1=C1, scalar2=C2, op0=Alu.mult, op1=Alu.add,
                )
                # Quake rsqrt (1 Newton iter) for the exp scale and den weights
                # (keeps the scalar engine FIFO free for the score exps)
                yi = small.tile([128, CC], I32, tag="qi1", name="yi")
                nc.vector.tensor_scalar(
                    out=yi[:], in0=ssT[:].bitcast(I32),
                    scalar1=1, scalar2=None, op0=Alu.arith_shift_right,
                )
                r0i = small.tile([128, CC], I32, tag="qi2", name="r0i")
                nc.vector.tensor_scalar(
                    out=r0i[:], in0=yi[:],
                    scalar1=-1, scalar2=MAGIC, op0=Alu.mult, op1=Alu.add,
                )
                t1 = small.tile([128, CC], F32, tag="qf1", name="t1")
                nc.vector.tensor_mul(t1[:], ssT[:], r0i[:].bitcast(F32))
                t2 = small.tile([128, CC], F32, tag="qf2", name="t2")
                nc.vector.tensor_mul(t2[:], t1[:], r0i[:].bitcast(F32))
                u = small.tile([128, CC], F32, tag="qf3", name="u")
                nc.vector.tensor_scalar(
                    out=u[:], in0=t2[:],
                    scalar1=-0.5, scalar2=1.5, op0=Alu.mult, op1=Alu.add,
                )
                rq = small.tile([128, CC], F32, tag="qf4", name="rq", bufs=5)
                nc.vector.tensor_mul(rq[:], r0i[:].bitcast(F32), u[:])
                scexp = stream.tile([128, CC], F32, tag="scexp", name="scexp", bufs=7)
                nc.vector.tensor_scalar_mul(scexp[:], rq[:], 1.0 / SW)
                sq = small.tile([128, CC], F32, tag="qf5", name="sq")
                nc.vector.tensor_mul(sq[:], ssT[:], rq[:])
                inv8 = stream.tile([128, CC, 16], F8, tag="inv8", name="inv8", bufs=7)
                nc.vector.tensor_scalar_mul(inv8[:, :, 0:1], sq[:], 1.0 / SQC)
                rqs[i] = rq
                scexps[i] = scexp
                lrcs[i] = lrc
                inv8s[i] = inv8

            def qprep(half, i):
                # rho row for the q tokens: PE mini-transpose of rho cols
                rhoR = small.tile([128, CC], F32R, tag="rhoR", name="rhoR")
                nc.vector.tensor_scalar_mul(rhoR[:], rqs[i][:], SQC)
                row_ps = psum_den.tile([1, S], F32, tag="den", name="row_ps")
                for kp in range(CC):
                    nc.tensor.matmul(
                        row_ps[0:1, kp * 128:(kp + 1) * 128],
                        rhoR[:, kp:kp + 1],
                        ident_sb[:],
                        start=True, stop=True,
                    )
                rho_row = small.tile([1, S], F32, tag="rrow", name="rho_row")
                nc.vector.tensor_scalar_mul(rho_row[:], row_ps[:], 1.0 / SW)
                rho_b = small.tile([128, S], F32, tag="rhob", name="rho_b")
                nc.gpsimd.partition_broadcast(rho_b[:], rho_row[:])
                qn8 = small.tile([128, CC, S], F8, tag="qn8", name="qn8")
                for co in range(CC):
                    q0_ps = psum_gen.tile([128, S], F32, tag="gen", name="q0_ps")
                    for t in range(2):
                        nc.tensor.matmul(
                            q0_ps[:],
                            wq8_sb[:, 2 * t:2 * t + 2, co * 128:(co + 1) * 128],
                            x8s[i][:, 2 * t:2 * t + 2, :],
                            start=(t == 0), stop=(t == 1), perf_mode=DR,
                        )
                    nc.vector.tensor_mul(qn8[:, co, :], q0_ps[:], rho_b[:])
                for ci in range(CC):
                    qk_ps = psum_gen.tile([128, S], F32, tag="gen", name="qk_ps")
                    for t in range(2):
                        nc.tensor.matmul(
                            qk_ps[:],
                            wk8_sb[:, 2 * t:2 * t + 2, ci * 128:(ci + 1) * 128],
                            qn8[:, 2 * t:2 * t + 2, :],
                            start=(t == 0), stop=(t == 1), perf_mode=DR,
                        )
                    nc.vector.tensor_scalar_add(
                        qk8_sb[:, ci, half * S:(half + 1) * S],
                        qk_ps[:], cvec_sb[:, ci:ci + 1],
                    )

            offs = {}
            p8s = {}
            qkcs = {}

            def prep_pair(j):
                off = nc.values_load(
                    qoff_sb[0:1, 2 * j:2 * j + 1],
                    engines=[mybir.EngineType.DVE],
                    min_val=0, max_val=S,
                    skip_runtime_bounds_check=True,
                )
                offs[j] = off
                # the dual-fp8 ISA check rejects register offsets on the
                # matmul rhs, so materialize this pair's q-half of qk with a
                # DVE copy (register offsets are fine there)
                qkc = stream.tile([128, CC, S], F8, tag="qkc", name="qkc", bufs=3)
                nc.vector.tensor_copy(qkc[:], qk8_sb[:, :, bass.ds(off, S)])
                qkcs[j] = qkc

            def scores_phase(i):
                qkc = qkcs[i // 2]
                p8t = stream.tile([128, CC, S], F8, tag="p8", name="p8t", bufs=4)
                for kp in range(CC):
                    s_ps = psum_sc.tile([128, S], F32, tag="sc", name="s_ps", bufs=3)
                    for t in range(2):
                        nc.tensor.matmul(
                            s_ps[:],
                            x8s[i][:, 2 * t:2 * t + 2, kp * 128:(kp + 1) * 128],
                            qkc[:, 2 * t:2 * t + 2, :],
                            start=(t == 0), stop=(t == 1), perf_mode=DR,
                        )
                    nc.scalar.activation(
                        p8t[:, kp, :], s_ps[:], Act.Exp,
                        bias=lrcs[i][:, kp:kp + 1],
                        scale=scexps[i][:, kp:kp + 1],
                    )
                p8s[i] = p8t

            def accum_pair(j):
                off = offs.pop(j)
                qkcs.pop(j)
                first = j < 2  # pairs 0/1 are the first touch of their q-half
                dn_ps = psum_den.tile([1, S], F32, tag="den", name="dn_ps")
                for sub in range(2):
                    i = 2 * j + sub
                    for t in range(2):
                        nc.tensor.matmul(
                            dn_ps[:],
                            inv8s[i][:, 2 * t:2 * t + 2, 0:1],
                            p8s[i][:, 2 * t:2 * t + 2, :],
                            start=(sub == 0 and t == 0),
                            stop=(sub == 1 and t == 1), perf_mode=DR,
                        )
                if first:
                    nc.vector.tensor_copy(den_sb[:, bass.ds(off, S)], dn_ps[:])
                else:
                    nc.vector.tensor_add(
                        den_sb[:, bass.ds(off, S)], den_sb[:, bass.ds(off, S)],
                        dn_ps[:],
                    )
                for ci in range(CC):
                    u_ps = psum_gen.tile([128, S], F32, tag="gen", name="u_ps")
                    for sub in range(2):
                        i = 2 * j + sub
                        for t in range(2):
                            nc.tensor.matmul(
                                u_ps[:],
                                xT8s[i][:, 2 * t:2 * t + 2, ci * 128:(ci + 1) * 128],
                                p8s[i][:, 2 * t:2 * t + 2, :],
                                start=(sub == 0 and t == 0),
                                stop=(sub == 1 and t == 1), perf_mode=DR,
                            )
                    if first:
                        nc.vector.tensor_copy(U_sb[:, ci, bass.ds(off, S)], u_ps[:])
                    else:
                        nc.vector.tensor_add(
                            U_sb[:, ci, bass.ds(off, S)],
                            U_sb[:, ci, bass.ds(off, S)],
                            u_ps[:],
                        )
                p8s.pop(2 * j)
                p8s.pop(2 * j + 1)

            u8s = {}
            u8s = {}

            def fin_pre(half):
                cols = half * S
                dent = small.tile([1, S], F32, tag="rrow", name="dent")
                nc.vector.tensor_scalar_mul(dent[:], den_sb[:, cols:cols + S], 1.0 / SU)
                denb = small.tile([128, S], F32, tag="denb", name="denb")
                nc.gpsimd.partition_broadcast(denb[:], dent[:])
                rdb = small.tile([128, S], F32, tag="rhob", name="rdb")
                nc.vector.reciprocal(rdb[:], denb[:])
                u8 = small.tile([128, CC, S], F8, tag="u8", name="u8")
                for ci in range(CC):
                    nc.vector.tensor_mul(u8[:, ci, :], U_sb[:, ci, cols:cols + S], rdb[:])
                u8s[half] = u8

            def fin_proj(half):
                cols = half * S
                u8 = u8s[half]
                for co in range(CC):
                    pr_ps = psum_gen.tile([128, S], F32, tag="gen", name="pr_ps")
                    for t in range(2):
                        nc.tensor.matmul(
                            pr_ps[:],
                            wvp8_sb[:, 2 * t:2 * t + 2, co * 128:(co + 1) * 128],
                            u8[:, 2 * t:2 * t + 2, :],
                            start=(t == 0), stop=(t == 1), perf_mode=DR,
                        )
                    res = small.tile([128, S], F32, tag="res", name="res")
                    nc.vector.scalar_tensor_tensor(
                        out=res[:],
                        in0=pr_ps[:],
                        scalar=invc_sb[:],
                        in1=xres_sb[:, co, cols:cols + S],
                        op0=Alu.mult,
                        op1=Alu.add,
                    )
                    nc.sync.dma_start(
                        out=out_d[co * 128:(co + 1) * 128, cols:cols + S], in_=res[:],
                    )

            # ---- schedule: stream loads first, then q-prep, then the
            # software-pipelined pair steps (scores of i+1 overlap the
            # exp/accumulate of i) ----
            NP = NSTEP // 2
            load_step(0)
            load_step(1)
            load_early_consts()
            load_step(2)
            load_step(3)
            stats_step(0)
            stats_step(1)
            stats_step(2)
            stats_step(3)
            qprep(0, 0)
            qprep(1, 2)
            prep_pair(0)
            prep_pair(1)
            scores_phase(0)
            scores_phase(1)
            for j in range(NP):
                if 2 * j + 4 < NSTEP:
                    load_step(2 * j + 4)
                    stats_step(2 * j + 4)
                    load_step(2 * j + 5)
                    stats_step(2 * j + 5)
                if j == 0:
                    load_late_consts()
                if j + 2 < NP:
                    prep_pair(j + 2)
                if 2 * j + 2 < NSTEP:
                    scores_phase(2 * j + 2)
                    scores_phase(2 * j + 3)
                accum_pair(j)
            fin_pre(0)
            fin_pre(1)
            fin_proj(0)
            fin_proj(1)

    nc.finalize()
    _cached["nc"] = nc
    return nc


def _q8(a):
    a = np.clip(np.asarray(a, np.float32), -240.0, 240.0)
    return a.astype(E4NP)


def _prep_inputs(x, gamma, wq, bq, wk, bk, wv, bv, wp, bp):
    x = np.asarray(x, np.float32)
    X = np.ascontiguousarray(x[0].reshape(C, F * HW))
    g = np.asarray(gamma, np.float32)
    wq = np.asarray(wq, np.float32)
    wk = np.asarray(wk, np.float32)
    wv = np.asarray(wv, np.float32)
    wp = np.asarray(wp, np.float32)
    bq = np.asarray(bq, np.float32)
    bv = np.asarray(bv, np.float32)
    bp = np.asarray(bp, np.float32)

    def pack_cols(a):
        # [C, n] -> [128, CC*n]: row p, col (ci, j) = a[ci*128+p, j]
        n = a.shape[1]
        return np.ascontiguousarray(
            a.reshape(CC, 128, n).transpose(1, 0, 2).reshape(128, CC * n)
        )

    wq8 = pack_cols(_q8(SW * (wq * g[None, :]).T))      # [cin, o]
    wk8 = pack_cols(_q8(SW * (wk * g[None, :])))        # [o, cin]
    wvp8 = pack_cols(_q8(SWVP * (wp @ (wv * g[None, :])).T))  # [cin, co]
    cvec = (SW * (wk * g[None, :]).T @ bq).astype(np.float32)
    cvec_p = np.ascontiguousarray(cvec.reshape(CC, 128).T)
    bvp = (bp + wp @ bv).astype(np.float32)

    X8 = _q8(X)                              # [C, seq] fp8
    XSQ8 = _q8(X8.astype(np.float32) ** 2)   # squares of the quantized x
    XT8 = np.ascontiguousarray(X8.T)         # [seq, C] fp8
    ident = np.eye(128, dtype=np.float32)

    common = {
        "wq8": wq8, "wk8": wk8, "wvp8": wvp8,
        "cvec": cvec_p,
        "ident": ident,
    }
    in_maps = []
    for j in range(F):
        p, h = j // 2, j % 2
        fa, fb = p, F - 1 - p
        ba, bb = 2 * fa + h, 2 * fb + h
        a_rest = [b for b in range(2 * fa + 2) if b != ba]
        b_rest = [b for b in range(2 * fb + 2) if b != bb]
        # step pairs share a q-half so U/den accumulate over pairs in PSUM
        steps = [ba, a_rest[0], bb, b_rest[0]] + a_rest[1:] + b_rest[1:]
        assert len(steps) == NSTEP
        qoffs = [0, 0, S, S] + [0] * (2 * fa) + [S] * (2 * fb)
        m = dict(common)
        # packed per-step tiles: [128, NSTEP*CC*S]
        m["x8"] = np.concatenate(
            [pack_cols(X8[:, b * S:(b + 1) * S]) for b in steps], axis=1
        )
        m["xsq8"] = np.concatenate(
            [pack_cols(XSQ8[:, b * S:(b + 1) * S][:, SSPERM]) for b in steps],
            axis=1,
        )
        # xT8 tile layout: row p, col (kp, c) = XT8[b*S + kp*128 + p, c]
        m["xT8"] = np.concatenate(
            [XT8[b * S:(b + 1) * S, :].reshape(CC, 128, C)
             .transpose(1, 0, 2).reshape(128, CC * C) for b in steps],
            axis=1,
        )
        m["xT8"] = np.ascontiguousarray(m["xT8"])
        m["qoff"] = np.asarray([qoffs], np.int32)
        xres = np.concatenate(
            [X[:, ba * S:(ba + 1) * S], X[:, bb * S:(bb + 1) * S]], axis=1
        ) + bvp[:, None]
        m["xres"] = pack_cols(xres.astype(np.float32))
        in_maps.append(m)
    return in_maps


def kernel(x, gamma, wq, bq, wk, bk, wv, bv, wp, bp, _trace=False):
    nc = _build()
    in_maps = _prep_inputs(x, gamma, wq, bq, wk, bk, wv, bv, wp, bp)
    kwargs = {}
    if _trace:
        kwargs = dict(trace=True, trace_cores=list(range(F)))
    r = run_bass_kernel_spmd(nc, in_maps, core_ids=list(range(F)), **kwargs)
    out = np.empty((1, C, F, HW), np.float32)
    for j in range(F):
        p, h = j // 2, j % 2
        fa, fb = p, F - 1 - p
        res = r.results[j]["out"]
        out[0, :, fa, h * S:h * S + S] = res[:, 0:S]
        out[0, :, fb, h * S:h * S + S] = res[:, S:Q]
    out = out.reshape(1, C, F, 32, 32)
    kernel._last_results = r
    return out


# revision 35
# speedup vs baseline: 1.4377x; 1.0002x over previous
"""Block-causal attention block (RMSnorm + QKV + frame-causal attention + proj)
on 8 TRN2 NeuronCores — fp8 DoubleRow + weight-folding edition.

Sharding: as the baseline — core j (p=j//2, h=j%2) owns query half-blocks
(frame p, col-half h) and (frame 7-p, col-half h); each core runs a uniform
stream of 18 (kv half-block, q-half) pair-steps (perfectly balanced since
2(p+1) + 2(8-p) = 18).

Algebraic folds (vs the baseline's per-step K/V projections):
  - K-fold: scores s[kv,q] = x_raw[:,kv]^T (Wk_fold q) — the K projection
    moves to the query side (one GEMM per q-half); the k-bias term is
    constant per query row and drops by softmax shift invariance.
  - V-fold: O = Wv_fold (sum_kv xn[:,kv] p[kv,q]) — the V projection moves
    to after the attention sum (one GEMM per q-half); bv folds through wp
    into the output bias since softmax rows sum to 1.
  - Norm-fold: RMS norm scalar rho[t] = sqrt(C)/||x_t|| is computed on the
    DVE (free-axis square-reduce of the transposed slab + Quake rsqrt) and
    applied as (a) the per-partition scale AP of the Exp activation on the
    kv side and (b) a broadcast row multiply on the q side. gamma folds
    into the weights host-side.

All matmuls run in fp8 e4m3 with perf_mode=DoubleRow (K=256 per
instruction, 2 fp8 MACs/cell/cycle). Weights are pre-scaled by 64 (16 for
Wv) host-side to sit in e4m3's normal range; the compensations fold into
the exp scale and two output-side constants. Residual x + bias stays f32.

Per-pair-step PE work: 8 DR matmuls (scores) + 8 (U accum) + 2 (den)
vs the baseline's 64+8 full-rate f32r matmuls per step.
"""

import sys

import numpy as np
import ml_dtypes

sys.path.insert(0, "/opt/trn_rl_repo")

import concourse.bacc as bacc
import concourse.bass as bass
import concourse.tile as tile
from concourse import mybir
from concourse.bass_utils import run_bass_kernel_spmd

C = 512
CC = C // 128          # 4 chunks of 128
F = 8                  # frames
HW = 1024              # tokens per frame
S = 512                # tokens per half-block / step
NSTEP = 18             # pair-steps per core (balanced)
Q = 1024               # queries per core (two half-blocks)
SW = 64.0              # Wq/Wk host scale
SWVP = 256.0           # fused Wv*Wp host scale
SU = 64.0              # den scale
SUB = 16.0             # unnormalized-U quantize scale
SQC = float(np.sqrt(C))
SCALE = 1.0 / SQC
MAGIC = 0x5F3759DF     # Quake rsqrt seed

F32 = mybir.dt.float32
F32R = mybir.dt.float32r
F8 = mybir.dt.float8e4
I32 = mybir.dt.int32
Act = mybir.ActivationFunctionType
Alu = mybir.AluOpType
DR = mybir.MatmulPerfMode.DoubleRow
E4NP = ml_dtypes.float8_e4m3
# column permutation for the stats stream: ss row element p*4+kp holds token
# kp*128+p, so one [1,512]->[128,4] DMA transposes it onto partitions
SSPERM = (np.arange(S) % CC) * 128 + np.arange(S) // CC

_cached = {}


def _build():
    if "nc" in _cached:
        return _cached["nc"]

    nc = bacc.Bacc()
    x8_d = nc.dram_tensor("x8", [128, NSTEP * CC * S], F8, kind="ExternalInput")
    xsq8_d = nc.dram_tensor("xsq8", [128, NSTEP * CC * S], F8, kind="ExternalInput")
    xT8_d = nc.dram_tensor("xT8", [128, NSTEP * CC * C], F8, kind="ExternalInput")
    qoff_d = nc.dram_tensor("qoff", [1, NSTEP], I32, kind="ExternalInput")
    wq8_d = nc.dram_tensor("wq8", [128, CC * C], F8, kind="ExternalInput")
    wk8_d = nc.dram_tensor("wk8", [128, CC * C], F8, kind="ExternalInput")
    wvp8_d = nc.dram_tensor("wvp8", [128, CC * C], F8, kind="ExternalInput")
    cvec_d = nc.dram_tensor("cvec", [128, CC], F32, kind="ExternalInput")
    xres_d = nc.dram_tensor("xres", [128, CC * Q], F32, kind="ExternalInput")
    ident_d = nc.dram_tensor("ident", [128, 128], F32, kind="ExternalInput")
    out_d = nc.dram_tensor("out", [C, Q], F32, kind="ExternalOutput")

    with tile.TileContext(nc) as tc:
        with (
            tc.tile_pool(name="const", bufs=1) as const,
            tc.tile_pool(name="persist", bufs=1) as persist,
            tc.tile_pool(name="stream", bufs=4) as stream,
            tc.tile_pool(name="small", bufs=2) as small,
            tc.tile_pool(name="psum_sc", bufs=2, space="PSUM") as psum_sc,
            tc.tile_pool(name="psum_gen", bufs=3, space="PSUM") as psum_gen,
            tc.tile_pool(name="psum_den", bufs=2, space="PSUM") as psum_den,
        ):
            # ---- constant tiles (loads for the q-side path emitted early;
            # wv/wp/xres deferred until after the stream is rolling) ----
            wq8_sb = const.tile([128, CC, C], F8, tag="wq8", name="wq8_sb")
            wk8_sb = const.tile([128, CC, C], F8, tag="wk8", name="wk8_sb")
            wvp8_sb = const.tile([128, CC, C], F8, tag="wvp8", name="wvp8_sb")
            cvec_sb = const.tile([128, CC], F32, tag="cvec", name="cvec_sb")
            ident_sb = const.tile([128, 128], F32R, tag="ident", name="ident_sb")
            qoff_sb = const.tile([1, NSTEP], I32, tag="qoff", name="qoff_sb")
            xres_sb = const.tile([128, CC, Q], F32, tag="xres", name="xres_sb")
            ones8 = const.tile([128, CC, 16], F8, tag="ones8", name="ones8")
            nc.vector.memset(ones8[:], 1.0)

            def load_early_consts():
                nc.sync.dma_start(out=wq8_sb[:], in_=wq8_d[:])
                nc.sync.dma_start(out=wk8_sb[:], in_=wk8_d[:])
                nc.sync.dma_start(out=cvec_sb[:], in_=cvec_d[:])
                nc.sync.dma_start(out=ident_sb[:], in_=ident_d[:].bitcast(F32R))
                nc.sync.dma_start(out=qoff_sb[:], in_=qoff_d[:])

            def load_late_consts():
                nc.sync.dma_start(out=wvp8_sb[:], in_=wvp8_d[:])
                nc.sync.dma_start(out=xres_sb[:], in_=xres_d[:])

            # ---- persistent accumulators (first pair-step writes them) ----
            qk8_sb = persist.tile([128, CC, Q], F8, tag="qk8", name="qk8_sb")
            U_sb = persist.tile([128, CC, Q], F32, tag="U", name="U_sb")
            den_sb = persist.tile([1, Q], F32, tag="den", name="den_sb")

            # ---- PE warmup: ~4.3us of back-to-back matmuls opens the HAM
            # clock gate (4/8 -> 8/8) before the real stream begins ----
            ones_f = const.tile([128, 1], F32, tag="ones_f", name="ones_f")
            nc.vector.memset(ones_f[:], 1.0)
            ones_r = const.tile([128, 1], F32R, tag="ones_r", name="ones_r")
            nc.vector.tensor_copy(ones_r[:], ones_f[:])
            warm_f = small.tile([128, S], F32, tag="warmf", name="warm_f", bufs=1)
            nc.vector.memset(warm_f[:], 0.0)
            warm_r = small.tile([128, S], F32R, tag="warmr", name="warm_r", bufs=1)
            nc.vector.tensor_copy(warm_r[:], warm_f[:])
            warm_ps = psum_den.tile([1, S], F32, tag="den", name="warm_ps")
            for wi in range(16):
                nc.tensor.matmul(
                    warm_ps[:], ones_r[:], warm_r[:],
                    start=(wi == 0), stop=(wi == 15),
                )

            x8s = {}
            xT8s = {}
            xsq8s = {}
            rqs = {}
            scexps = {}
            lrcs = {}
            inv8s = {}
            # bit-trick log constants: ln(ss) ~= ln2*(bits*2^-23 - 127 + mu)
            C1 = float(-np.log(2.0) / (1 << 24))
            C2 = float(0.5 * np.log(2.0) * (127.0 - 0.0450466) + 0.5 * np.log(C))

            def load_step(i):
                W = CC * S
                xsq8 = stream.tile([128, CC, S], F8, tag="xsq8", name="xsq8", bufs=7)
                nc.sync.dma_start(out=xsq8[:], in_=xsq8_d[:, i * W:(i + 1) * W])
                x8t = stream.tile([128, CC, S], F8, tag="x8", name="x8t", bufs=7)
                nc.sync.dma_start(out=x8t[:], in_=x8_d[:, i * W:(i + 1) * W])
                xT8t = stream.tile([128, CC, C], F8, tag="xT8", name="xT8t", bufs=7)
                nc.sync.dma_start(out=xT8t[:], in_=xT8_d[:, i * W:(i + 1) * W])
                x8s[i] = x8t
                xT8s[i] = xT8t
                xsq8s[i] = xsq8

            def stats_step(i):
                # squares precomputed host-side; channel-sum on PE
                xsq8 = xsq8s[i]
                ssr_ps = psum_den.tile([1, S], F32, tag="den", name="ssr_ps")
                for t in range(2):
                    nc.tensor.matmul(
                        ssr_ps[:],
                        ones8[:, 2 * t:2 * t + 2, 0:1],
                        xsq8[:, 2 * t:2 * t + 2, :],
                        start=(t == 0), stop=(t == 1), perf_mode=DR,
                    )
                ss_row = small.tile([1, S], F32, tag="ssrow", name="ss_row", bufs=3)
                nc.vector.tensor_copy(ss_row[:], ssr_ps[:])
                # xsq8 columns are host-permuted so this single strided DMA
                # lands the per-token row transposed onto partitions
                ssT = small.tile([128, CC], F32, tag="ssT", name="ssT", bufs=3)
                nc.sync.dma_start(out=ssT[:], in_=ss_row[0:1, :])
                # LrC = ln(rho) = ln(sqrt(C)) - 0.5*ln(ss), via exponent-bits log
                bitsf = small.tile([128, CC], F32, tag="bitsf", name="bitsf")
                nc.vector.tensor_copy(bitsf[:], ssT[:].bitcast(I32))
                lrc = stream.tile([128, CC], F32, tag="lrc", name="lrc", bufs=7)
                nc.vector.tensor_scalar(
                    out=lrc[:], in0=bitsf[:],
                    scalar1=C1, scalar2=C2, op0=Alu.mult, op1=Alu.add,
                )
                # Quake rsqrt (1 Newton iter) for the exp scale and den weights
                # (keeps the scalar engine FIFO free for the score exps)
                yi = small.tile([128, CC], I32, tag="qi1", name="yi")
                nc.vector.tensor_scalar(
                    out=yi[:], in0=ssT[:].bitcast(I32),
                    scalar1=1, scalar2=None, op0=Alu.arith_shift_right,
                )
                r0i = small.tile([128, CC], I32, tag="qi2", name="r0i")
                nc.vector.tensor_scalar(
                    out=r0i[:], in0=yi[:],
                    scalar1=-1, scalar2=MAGIC, op0=Alu.mult, op1=Alu.add,
                )
                t1 = small.tile([128, CC], F32, tag="qf1", name="t1")
                nc.vector.tensor_mul(t1[:], ssT[:], r0i[:].bitcast(F32))
                t2 = small.tile([128, CC], F32, tag="qf2", name="t2")
                nc.vector.tensor_mul(t2[:], t1[:], r0i[:].bitcast(F32))
                u = small.tile([128, CC], F32, tag="qf3", name="u")
                nc.vector.tensor_scalar(
                    out=u[:], in0=t2[:],
                    scalar1=-0.5, scalar2=1.5, op0=Alu.mult, op1=Alu.add,
                )
                rq = small.tile([128, CC], F32, tag="qf4", name="rq", bufs=5)
                nc.vector.tensor_mul(rq[:], r0i[:].bitcast(F32), u[:])
                scexp = stream.tile([128, CC], F32, tag="scexp", name="scexp", bufs=7)
                nc.vector.tensor_scalar_mul(scexp[:], rq[:], 1.0 / SW)
                sq = small.tile([128, CC], F32, tag="qf5", name="sq")
                nc.vector.tensor_mul(sq[:], ssT[:], rq[:])
                inv8 = stream.tile([128, CC, 16], F8, tag="inv8", name="inv8", bufs=7)
                nc.vector.tensor_scalar_mul(inv8[:, :, 0:1], sq[:], 1.0 / SQC)
                rqs[i] = rq
                scexps[i] = scexp
                lrcs[i] = lrc
                inv8s[i] = inv8

            rhobs = {}

            def qprep_rho(half, i):
                # rho row for the q tokens: PE mini-transpose of rho cols
                rhoR = small.tile([128, CC], F32R, tag="rhoR", name="rhoR")
                nc.vector.tensor_scalar_mul(rhoR[:], rqs[i][:], SQC)
                row_ps = psum_den.tile([1, S], F32, tag="den", name="row_ps")
                for kp in range(CC):
                    nc.tensor.matmul(
                        row_ps[0:1, kp * 128:(kp + 1) * 128],
                        rhoR[:, kp:kp + 1],
                        ident_sb[:],
                        start=True, stop=True,
                    )
                rho_row = small.tile([1, S], F32, tag="rrow", name="rho_row")
                nc.vector.tensor_scalar_mul(rho_row[:], row_ps[:], 1.0 / SW)
                rho_b = small.tile([128, S], F32, tag="rhob", name="rho_b")
                nc.gpsimd.partition_broadcast(rho_b[:], rho_row[:])
                rhobs[half] = rho_b

            def qprep_rest(half, i):
                qn8 = small.tile([128, CC, S], F8, tag="qn8", name="qn8")
                for co in range(CC):
                    q0_ps = psum_gen.tile([128, S], F32, tag="gen", name="q0_ps")
                    for t in range(2):
                        nc.tensor.matmul(
                            q0_ps[:],
                            wq8_sb[:, 2 * t:2 * t + 2, co * 128:(co + 1) * 128],
                            x8s[i][:, 2 * t:2 * t + 2, :],
                            start=(t == 0), stop=(t == 1), perf_mode=DR,
                        )
                    nc.vector.tensor_mul(qn8[:, co, :], q0_ps[:], rhobs[half][:])
                for ci in range(CC):
                    qk_ps = psum_gen.tile([128, S], F32, tag="gen", name="qk_ps")
                    for t in range(2):
                        nc.tensor.matmul(
                            qk_ps[:],
                            wk8_sb[:, 2 * t:2 * t + 2, ci * 128:(ci + 1) * 128],
                            qn8[:, 2 * t:2 * t + 2, :],
                            start=(t == 0), stop=(t == 1), perf_mode=DR,
                        )
                    nc.vector.tensor_scalar_add(
                        qk8_sb[:, ci, half * S:(half + 1) * S],
                        qk_ps[:], cvec_sb[:, ci:ci + 1],
                    )

            offs = {}
            p8s = {}
            qkcs = {}

            def prep_pair(j):
                off = nc.values_load(
                    qoff_sb[0:1, 2 * j:2 * j + 1],
                    engines=[mybir.EngineType.DVE],
                    min_val=0, max_val=S,
                    skip_runtime_bounds_check=True,
                )
                offs[j] = off
                # the dual-fp8 ISA check rejects register offsets on the
                # matmul rhs, so materialize this pair's q-half of qk with a
                # DVE copy (register offsets are fine there)
                qkc = stream.tile([128, CC, S], F8, tag="qkc", name="qkc", bufs=3)
                nc.vector.tensor_copy(qkc[:], qk8_sb[:, :, bass.ds(off, S)])
                qkcs[j] = qkc

            def scores_phase(i):
                qkc = qkcs[i // 2]
                p8t = stream.tile([128, CC, S], F8, tag="p8", name="p8t", bufs=4)
                for kp in range(CC):
                    s_ps = psum_sc.tile([128, S], F32, tag="sc", name="s_ps", bufs=3)
                    for t in range(2):
                        nc.tensor.matmul(
                            s_ps[:],
                            x8s[i][:, 2 * t:2 * t + 2, kp * 128:(kp + 1) * 128],
                            qkc[:, 2 * t:2 * t + 2, :],
                            start=(t == 0), stop=(t == 1), perf_mode=DR,
                        )
                    nc.scalar.activation(
                        p8t[:, kp, :], s_ps[:], Act.Exp,
                        bias=lrcs[i][:, kp:kp + 1],
                        scale=scexps[i][:, kp:kp + 1],
                    )
                p8s[i] = p8t

            def accum_pair(j):
                off = offs.pop(j)
                qkcs.pop(j)
                first = j < 2  # pairs 0/1 are the first touch of their q-half
                dn_ps = psum_den.tile([1, S], F32, tag="den", name="dn_ps")
                for sub in range(2):
                    i = 2 * j + sub
                    for t in range(2):
                        nc.tensor.matmul(
                            dn_ps[:],
                            inv8s[i][:, 2 * t:2 * t + 2, 0:1],
                            p8s[i][:, 2 * t:2 * t + 2, :],
                            start=(sub == 0 and t == 0),
                            stop=(sub == 1 and t == 1), perf_mode=DR,
                        )
                if first:
                    nc.vector.tensor_copy(den_sb[:, bass.ds(off, S)], dn_ps[:])
                else:
                    nc.vector.tensor_add(
                        den_sb[:, bass.ds(off, S)], den_sb[:, bass.ds(off, S)],
                        dn_ps[:],
                    )
                for ci in range(CC):
                    u_ps = psum_gen.tile([128, S], F32, tag="gen", name="u_ps")
                    for sub in range(2):
                        i = 2 * j + sub
                        for t in range(2):
                            nc.tensor.matmul(
                                u_ps[:],
                                xT8s[i][:, 2 * t:2 * t + 2, ci * 128:(ci + 1) * 128],
                                p8s[i][:, 2 * t:2 * t + 2, :],
                                start=(sub == 0 and t == 0),
                                stop=(sub == 1 and t == 1), perf_mode=DR,
                            )
                    if first:
                        nc.vector.tensor_copy(U_sb[:, ci, bass.ds(off, S)], u_ps[:])
                    else:
                        nc.vector.tensor_add(
                            U_sb[:, ci, bass.ds(off, S)],
                            U_sb[:, ci, bass.ds(off, S)],
                            u_ps[:],
                        )
                p8s.pop(2 * j)
                p8s.pop(2 * j + 1)

            u8s = {}
            u8s = {}
            rdbs = {}

            def fin_pre(half):
                cols = half * S
                # quantize U without waiting for den (normalize after the GEMM)
                u8 = small.tile([128, CC, S], F8, tag="u8", name="u8")
                for ci in range(CC):
                    nc.vector.tensor_scalar_mul(
                        u8[:, ci, :], U_sb[:, ci, cols:cols + S], 1.0 / SUB,
                    )
                u8s[half] = u8
                dent = small.tile([1, S], F32, tag="rrow", name="dent")
                nc.vector.tensor_scalar_mul(
                    dent[:], den_sb[:, cols:cols + S], SWVP / SUB,
                )
                denb = small.tile([128, S], F32, tag="denb", name="denb")
                nc.gpsimd.partition_broadcast(denb[:], dent[:])
                rdb = small.tile([128, S], F32, tag="rhob", name="rdb")
                nc.vector.reciprocal(rdb[:], denb[:])
                rdbs[half] = rdb

            def fin_proj(half):
                cols = half * S
                u8 = u8s[half]
                rdb = rdbs[half]
                for co in range(CC):
                    pr_ps = psum_gen.tile([128, S], F32, tag="gen", name="pr_ps")
                    for t in range(2):
                        nc.tensor.matmul(
                            pr_ps[:],
                            wvp8_sb[:, 2 * t:2 * t + 2, co * 128:(co + 1) * 128],
                            u8[:, 2 * t:2 * t + 2, :],
                            start=(t == 0), stop=(t == 1), perf_mode=DR,
                        )
                    prn = small.tile([128, S], F32, tag="prn", name="prn")
                    nc.vector.tensor_mul(prn[:], pr_ps[:], rdb[:])
                    res = small.tile([128, S], F32, tag="res", name="res")
                    nc.vector.tensor_add(res[:], prn[:], xres_sb[:, co, cols:cols + S])
                    nc.sync.dma_start(
                        out=out_d[co * 128:(co + 1) * 128, cols:cols + S], in_=res[:],
                    )

            # ---- schedule: stream loads first, then q-prep, then the
            # software-pipelined pair steps (scores of i+1 overlap the
            # exp/accumulate of i) ----
            NP = NSTEP // 2
            load_step(0)
            load_step(1)
            load_early_consts()
            load_step(2)
            load_step(3)
            stats_step(0)
            stats_step(1)
            stats_step(2)
            stats_step(3)
            qprep_rho(0, 0)
            qprep_rho(1, 2)
            qprep_rest(0, 0)
            qprep_rest(1, 2)
            prep_pair(0)
            prep_pair(1)
            scores_phase(0)
            scores_phase(1)
            for j in range(NP):
                if 2 * j + 4 < NSTEP:
                    load_step(2 * j + 4)
                    stats_step(2 * j + 4)
                    load_step(2 * j + 5)
                    stats_step(2 * j + 5)
                if j == 0:
                    load_late_consts()
                if j + 2 < NP:
                    prep_pair(j + 2)
                if 2 * j + 2 < NSTEP:
                    scores_phase(2 * j + 2)
                    scores_phase(2 * j + 3)
                accum_pair(j)
            fin_pre(0)
            fin_pre(1)
            fin_proj(0)
            fin_proj(1)

    nc.finalize()
    _cached["nc"] = nc
    return nc


def _q8(a):
    a = np.clip(np.asarray(a, np.float32), -240.0, 240.0)
    return a.astype(E4NP)


def _prep_inputs(x, gamma, wq, bq, wk, bk, wv, bv, wp, bp):
    x = np.asarray(x, np.float32)
    X = np.ascontiguousarray(x[0].reshape(C, F * HW))
    g = np.asarray(gamma, np.float32)
    wq = np.asarray(wq, np.float32)
    wk = np.asarray(wk, np.float32)
    wv = np.asarray(wv, np.float32)
    wp = np.asarray(wp, np.float32)
    bq = np.asarray(bq, np.float32)
    bv = np.asarray(bv, np.float32)
    bp = np.asarray(bp, np.float32)

    def pack_cols(a):
        # [C, n] -> [128, CC*n]: row p, col (ci, j) = a[ci*128+p, j]
        n = a.shape[1]
        return np.ascontiguousarray(
            a.reshape(CC, 128, n).transpose(1, 0, 2).reshape(128, CC * n)
        )

    wq8 = pack_cols(_q8(SW * (wq * g[None, :]).T))      # [cin, o]
    wk8 = pack_cols(_q8(SW * (wk * g[None, :])))        # [o, cin]
    wvp8 = pack_cols(_q8(SWVP * (wp @ (wv * g[None, :])).T))  # [cin, co]
    cvec = (SW * (wk * g[None, :]).T @ bq).astype(np.float32)
    cvec_p = np.ascontiguousarray(cvec.reshape(CC, 128).T)
    bvp = (bp + wp @ bv).astype(np.float32)

    X8 = _q8(X)                              # [C, seq] fp8
    XSQ8 = _q8(X8.astype(np.float32) ** 2)   # squares of the quantized x
    XT8 = np.ascontiguousarray(X8.T)         # [seq, C] fp8
    ident = np.eye(128, dtype=np.float32)

    common = {
        "wq8": wq8, "wk8": wk8, "wvp8": wvp8,
        "cvec": cvec_p,
        "ident": ident,
    }
    in_maps = []
    for j in range(F):
        p, h = j // 2, j % 2
        fa, fb = p, F - 1 - p
        ba, bb = 2 * fa + h, 2 * fb + h
        a_rest = [b for b in range(2 * fa + 2) if b != ba]
        b_rest = [b for b in range(2 * fb + 2) if b != bb]
        # step pairs share a q-half so U/den accumulate over pairs in PSUM
        steps = [ba, a_rest[0], bb, b_rest[0]] + a_rest[1:] + b_rest[1:]
        assert len(steps) == NSTEP
        qoffs = [0, 0, S, S] + [0] * (2 * fa) + [S] * (2 * fb)
        m = dict(common)
        # packed per-step tiles: [128, NSTEP*CC*S]
        m["x8"] = np.concatenate(
            [pack_cols(X8[:, b * S:(b + 1) * S]) for b in steps], axis=1
        )
        m["xsq8"] = np.concatenate(
            [pack_cols(XSQ8[:, b * S:(b + 1) * S][:, SSPERM]) for b in steps],
            axis=1,
        )
        # xT8 tile layout: row p, col (kp, c) = XT8[b*S + kp*128 + p, c]
        m["xT8"] = np.concatenate(
            [XT8[b * S:(b + 1) * S, :].reshape(CC, 128, C)
             .transpose(1, 0, 2).reshape(128, CC * C) for b in steps],
            axis=1,
        )
        m["xT8"] = np.ascontiguousarray(m["xT8"])
        m["qoff"] = np.asarray([qoffs], np.int32)
        xres = np.concatenate(
            [X[:, ba * S:(ba + 1) * S], X[:, bb * S:(bb + 1) * S]], axis=1
        ) + bvp[:, None]
        m["xres"] = pack_cols(xres.astype(np.float32))
        in_maps.append(m)
    return in_maps


def kernel(x, gamma, wq, bq, wk, bk, wv, bv, wp, bp, _trace=False):
    nc = _build()
    in_maps = _prep_inputs(x, gamma, wq, bq, wk, bk, wv, bv, wp, bp)
    kwargs = {}
    if _trace:
        kwargs = dict(trace=True, trace_cores=list(range(F)))
    r = run_bass_kernel_spmd(nc, in_maps, core_ids=list(range(F)), **kwargs)
    out = np.empty((1, C, F, HW), np.float32)
    for j in range(F):
        p, h = j // 2, j % 2
        fa, fb = p, F - 1 - p
        res = r.results[j]["out"]
        out[0, :, fa, h * S:h * S + S] = res[:, 0:S]
        out[0, :, fb, h * S:h * S + S] = res[:, S:Q]
    out = out.reshape(1, C, F, 32, 32)
    kernel._last_results = r
    return out
